# revision 19
# baseline (speedup 1.0000x reference)
"""Trainium2 Bass kernel for nn_GTGModule (GTG message passing + MLP heads).

Self-contained: accepts FULL inputs (as produced by the problem's
setup_inputs), shards across 8 NeuronCores internally, returns FULL outputs
(y_pred, y_true, mask).

Strategy (single SPMD launch, 8 cores, 3 AllGathers):
  - batch-sharded feature pooling (the ~1GB memory-bound part)
  - host pre-permutes the batch so labelled rows are [0, nlab) => the
    labelled x labelled block of the affinity matrix is tile-contiguous
  - each of cores 0-3 runs one branch's full B x B GTG; core 4 runs the
    embedds GTG; selection is done with per-core 0/1 mask inputs (pure SPMD)
  - MLP runs with transposed activations so BatchNorm is a free-dim reduce
"""

import numpy as np
import ml_dtypes

import concourse.bass as bass
import concourse.bacc as bacc
import concourse.mybir as mybir
from concourse import tile
from concourse.bass_utils import run_bass_kernel_spmd

F32 = mybir.dt.float32
BF16 = mybir.dt.bfloat16
ALU = mybir.AluOpType
ACTF = mybir.ActivationFunctionType
AX = mybir.AxisListType

B = 2048
NC_CLS = 10
NCORES = 8
BLOC = B // NCORES          # 256 batch rows per core
CHS = [64, 128, 256, 512]
FSS = [32, 16, 8, 4]
INTERM = 128
MAXIT = 30
TOL = 1e-3
BN_EPS = 1e-5
NT = B // 128               # 16 row blocks
DMA_CHUNK = 4096            # free elems per pooling DMA tile (16KB/partition)


# --------------------------------------------------------------------------
# device program
# --------------------------------------------------------------------------

def _build_program(nlab, maxit=MAXIT, solo=False):
    nc = bacc.Bacc("TRN2", num_devices=NCORES)

    # ---------------- inputs ----------------
    feats = [nc.dram_tensor(f"f{i}", [BLOC, CHS[i] * FSS[i] * FSS[i]], F32,
                            kind="ExternalInput") for i in range(4)]
    ws = [nc.dram_tensor(f"w{i}", [CHS[i], INTERM], F32, kind="ExternalInput")
          for i in range(4)]
    bs = [nc.dram_tensor(f"b{i}", [INTERM, 1], F32, kind="ExternalInput")
          for i in range(4)]
    ebase = nc.dram_tensor("ebase", [4, 128, B], BF16, kind="ExternalInput")
    amask_in = nc.dram_tensor("amask", [128, 5], F32, kind="ExternalInput")
    x0_in = nc.dram_tensor("x0", [128, NT * NC_CLS], F32, kind="ExternalInput")
    ident_in = nc.dram_tensor("ident", [128, 128], F32, kind="ExternalInput")
    w1a = nc.dram_tensor("mw1a", [128, 129], F32, kind="ExternalInput")
    w1b = nc.dram_tensor("mw1b", [1, 129], F32, kind="ExternalInput")
    w2a = nc.dram_tensor("mw2a", [128, 64], F32, kind="ExternalInput")
    w2b = nc.dram_tensor("mw2b", [1, 64], F32, kind="ExternalInput")
    w3 = nc.dram_tensor("mw3", [64, 64], F32, kind="ExternalInput")
    w4 = nc.dram_tensor("mw4", [64, 32], F32, kind="ExternalInput")
    w5 = nc.dram_tensor("mw5", [32, 32], F32, kind="ExternalInput")
    g1a = nc.dram_tensor("g1a", [128, 1], F32, kind="ExternalInput")
    g1b = nc.dram_tensor("g1b", [1, 1], F32, kind="ExternalInput")
    be1a = nc.dram_tensor("be1a", [128, 1], F32, kind="ExternalInput")
    be1b = nc.dram_tensor("be1b", [1, 1], F32, kind="ExternalInput")
    g2 = nc.dram_tensor("g2", [64, 1], F32, kind="ExternalInput")
    be2 = nc.dram_tensor("be2", [64, 1], F32, kind="ExternalInput")
    g3 = nc.dram_tensor("g3", [64, 1], F32, kind="ExternalInput")
    be3 = nc.dram_tensor("be3", [64, 1], F32, kind="ExternalInput")
    g4 = nc.dram_tensor("g4", [32, 1], F32, kind="ExternalInput")
    be4 = nc.dram_tensor("be4", [32, 1], F32, kind="ExternalInput")
    g5 = nc.dram_tensor("g5", [32, 1], F32, kind="ExternalInput")
    be5 = nc.dram_tensor("be5", [32, 1], F32, kind="ExternalInput")
    finw = nc.dram_tensor("finw", [128, 1], F32, kind="ExternalInput")
    finb = nc.dram_tensor("finb", [1, 1], F32, kind="ExternalInput")

    # ---------------- outputs ----------------
    ypred_out = nc.dram_tensor("y_pred", [B], F32, kind="ExternalOutput")
    q_out = nc.dram_tensor("q_out", [B], F32, kind="ExternalOutput")

    # ---------------- internal DRAM ----------------
    ag1n_in = nc.dram_tensor("ag1n_in", [4, 128, BLOC], BF16, kind="Internal")
    ag1n_out = nc.dram_tensor("ag1n_out", [NCORES, 4, 128, BLOC], BF16,
                              kind="Internal", addr_space="Shared")
    ag1r_in = nc.dram_tensor("ag1r_in", [4, 128, BLOC], F32, kind="Internal")
    ag1r_out = nc.dram_tensor("ag1r_out", [NCORES, 4, 128, BLOC], F32,
                              kind="Internal", addr_space="Shared")
    ag2_in = nc.dram_tensor("ag2_in", [32, B], F32, kind="Internal")
    ag2_out = nc.dram_tensor("ag2_out", [NCORES, 32, B], F32,
                             kind="Internal", addr_space="Shared")
    eraw_dram = nc.dram_tensor("eraw_dram", [128, B], F32, kind="Internal")
    q_dram = nc.dram_tensor("q_dram", [B], F32, kind="Internal")
    rg = [list(range(NCORES))]

    with tile.TileContext(nc) as tc:
        with tc.tile_pool(name="persist", bufs=1) as pp, \
             tc.tile_pool(name="a16", bufs=1) as a16pool, \
             tc.tile_pool(name="psmall", bufs=2, space="PSUM") as psmall:

            ident = pp.tile([128, 128], F32, tag="ident")
            nc.sync.dma_start(ident[:], ident_in[:, :])
            amask = pp.tile([128, 5], F32, tag="amask")
            nc.sync.dma_start(amask[:], amask_in[:, :])
            ones_col = pp.tile([128, 1], F32, tag="ones_col")
            nc.vector.memset(ones_col[:], 1.0)
            ones_row = pp.tile([1, 128], F32, tag="ones_row")
            nc.vector.memset(ones_row[:], 1.0)
            ones10 = pp.tile([10, 1], F32, tag="ones10")
            nc.vector.memset(ones10[:], 1.0)

            # ============= phase 1: pooling + branch embeddings =============
            with tc.tile_pool(name="pool_dma", bufs=4) as fpool, \
                 tc.tile_pool(name="pool_rm", bufs=2) as prpool, \
                 tc.tile_pool(name="pool_w", bufs=1) as wpool, \
                 tc.tile_pool(name="pooledT", bufs=1) as ptpool, \
                 tc.tile_pool(name="emb", bufs=1) as embpool, \
                 tc.tile_pool(name="pool_ps", bufs=2, space="PSUM") as ppsum, \
                 tc.tile_pool(name="emb_ps", bufs=2, space="PSUM") as epsum:
                embraw = []   # (128, BLOC) f32, per branch
                ntpart = []   # (128, BLOC) bf16, per branch (row-normalized)
                for i in range(4):
                    CH, S = CHS[i], FSS[i] * FSS[i]
                    row = CH * S
                    nchunks = row // DMA_CHUNK
                    chpc = DMA_CHUNK // S          # channels per chunk
                    nkchunk = (CH + 127) // 128    # partition chunks of pooledT
                    ptiles = [ptpool.tile([128, BLOC], F32, tag=f"pt{i}_{j}",
                                          name=f"pt{i}_{j}")
                              for j in range(nkchunk)]
                    for bt in range(BLOC // 128):
                        pr = prpool.tile([128, CH], F32, tag="poolrm")
                        for c in range(nchunks):
                            ft = fpool.tile([128, DMA_CHUNK], F32, tag="ftile")
                            nc.sync.dma_start(
                                ft[:],
                                feats[i][128 * bt:128 * (bt + 1),
                                         DMA_CHUNK * c:DMA_CHUNK * (c + 1)])
                            nc.vector.tensor_reduce(
                                pr[:, chpc * c:chpc * (c + 1)],
                                ft[:].rearrange("p (ch s) -> p ch s", s=S),
                                axis=AX.X, op=ALU.add)
                        # transpose (128, CH) -> (CH, 128) into pooledT chunks
                        for j in range(nkchunk):
                            w = min(128, CH - 128 * j)
                            tp = ppsum.tile([128, 128], F32, tag="tpsum")
                            nc.tensor.transpose(
                                tp[:w, :], pr[:, 128 * j:128 * j + w], ident[:])
                            nc.scalar.copy(
                                ptiles[j][:w, 128 * bt:128 * (bt + 1)], tp[:w, :])
                    # emb = relu(W.T @ pooledT + bias): out (128, BLOC)
                    kp = min(128, CH)
                    wsb = wpool.tile([128, nkchunk * INTERM], F32, tag=f"wsb{i}")
                    nc.sync.dma_start(
                        wsb[:kp, :].rearrange("p (k m) -> p k m", m=INTERM),
                        ws[i][:, :].rearrange("(k p) m -> p k m", p=kp))
                    bsb = wpool.tile([128, 1], F32, tag=f"bsb{i}")
                    nc.sync.dma_start(bsb[:], bs[i][:, :])
                    eps = epsum.tile([128, BLOC], F32, tag="embps")
                    for j in range(nkchunk):
                        w = min(128, CH - 128 * j)
                        nc.tensor.matmul(
                            eps[:, :], wsb[:w, INTERM * j:INTERM * (j + 1)],
                            ptiles[j][:w, :],
                            start=(j == 0), stop=(j == nkchunk - 1))
                    er = embpool.tile([128, BLOC], F32, tag=f"embraw{i}")
                    nc.scalar.activation(er[:], eps[:], ACTF.Relu, bias=bsb[:, 0:1])
                    embraw.append(er)
                    # row-normalize (norm over the 128 channels = partition dim)
                    sq = prpool.tile([128, BLOC], F32, tag="sqscratch")
                    nc.scalar.activation(sq[:], er[:], ACTF.Square)
                    n2 = psmall.tile([1, BLOC], F32, tag="ps")
                    nc.tensor.matmul(n2[:], ones_col[:], sq[:], start=True, stop=True)
                    nrm = prpool.tile([1, BLOC], F32, tag="nrm")
                    nc.scalar.activation(nrm[:], n2[:], ACTF.Sqrt)
                    nc.vector.tensor_scalar(nrm[:], nrm[:], 1e-12, None, op0=ALU.max)
                    rn = prpool.tile([1, BLOC], F32, tag="rn")
                    nc.vector.reciprocal(rn[:], nrm[:])
                    rnb = psmall.tile([128, BLOC], F32, tag="ps")
                    nc.tensor.matmul(rnb[:], ones_row[:], rn[:], start=True, stop=True)
                    nt_ = embpool.tile([128, BLOC], BF16, tag=f"ntpart{i}")
                    nc.vector.tensor_tensor(nt_[:], er[:], rnb[:], op=ALU.mult)
                    ntpart.append(nt_)

                # ===== phase 2: AllGather emb parts =====
                for i in range(4):
                    nc.sync.dma_start(ag1n_in[i, :, :], ntpart[i][:])
                    nc.sync.dma_start(ag1r_in[i, :, :], embraw[i][:])
            if solo:
                nc.sync.dma_start(ag1n_out[0, :, :, :], ag1n_in[:, :, :])
                nc.sync.dma_start(ag1r_out[0, :, :, :], ag1r_in[:, :, :])
            else:
                nc.gpsimd.collective_compute(
                    "AllGather", ALU.bypass, replica_groups=rg,
                    ins=[ag1n_in[:, :, :].opt()],
                    outs=[ag1n_out[:, :, :, :].opt()])
                nc.gpsimd.collective_compute(
                    "AllGather", ALU.bypass, replica_groups=rg,
                    ins=[ag1r_in[:, :, :].opt()],
                    outs=[ag1r_out[:, :, :, :].opt()])

            # ============= phase 3: blend per-core E, build A =============
            a16 = [a16pool.tile([128, B], BF16, tag=f"a16_{t}", name=f"a16_{t}")
                   for t in range(NT)]
            rowsums = pp.tile([128, 4 * NT], F32, tag="rowsums")
            if True:
                with tc.tile_pool(name="enorm", bufs=1) as enpool:
                    enorm = [enpool.tile([128, B], BF16, tag=f"en{q}", name=f"en{q}")
                             for q in range(4)]
                    with tc.tile_pool(name="blend", bufs=2) as blpool:
                        # raw E blend (chunk 0 only; branch cores only)
                        eraw = blpool.tile([128, B], F32, tag="erawblend")
                        for i in range(4):
                            rb = blpool.tile([128, B], F32, tag="ldtile")
                            nc.sync.dma_start(
                                rb[:].rearrange("p (r b) -> p r b", b=BLOC),
                                ag1r_out[:, i, :, :].rearrange("r p b -> p r b"))
                            if i == 0:
                                nc.vector.tensor_scalar(
                                    eraw[:], rb[:], amask[:, 0:1], None,
                                    op0=ALU.mult)
                            else:
                                nc.vector.scalar_tensor_tensor(
                                    eraw[:], rb[:], amask[:, i:i + 1], eraw[:],
                                    op0=ALU.mult, op1=ALU.add)
                        nc.sync.dma_start(eraw_dram[:, :], eraw[:])
                        # normalized E blend: chunk 0 from branches + ebase
                        for i in range(4):
                            nb = blpool.tile([128, B], BF16, tag="ldtile")
                            nc.sync.dma_start(
                                nb[:].rearrange("p (r b) -> p r b", b=BLOC),
                                ag1n_out[:, i, :, :].rearrange("r p b -> p r b"))
                            if i == 0:
                                nc.vector.tensor_scalar(
                                    enorm[0][:], nb[:], amask[:, 0:1], None,
                                    op0=ALU.mult)
                            else:
                                nc.vector.scalar_tensor_tensor(
                                    enorm[0][:], nb[:], amask[:, i:i + 1],
                                    enorm[0][:], op0=ALU.mult, op1=ALU.add)
                        eb = blpool.tile([128, B], BF16, tag="ldtile")
                        nc.sync.dma_start(eb[:], ebase[0, :, :])
                        nc.vector.scalar_tensor_tensor(
                            enorm[0][:], eb[:], amask[:, 4:5], enorm[0][:],
                            op0=ALU.mult, op1=ALU.add)
                        for qch in range(1, 4):
                            ebq = blpool.tile([128, B], BF16, tag="ldtile")
                            nc.sync.dma_start(ebq[:], ebase[qch, :, :])
                            nc.vector.tensor_scalar(
                                enorm[qch][:], ebq[:], amask[:, 4:5], None,
                                op0=ALU.mult)

                    # A = relu(E^T E) with fused fp32 row-sums, stored bf16
                    with tc.tile_pool(name="aps", bufs=4, space="PSUM") as apsum:
                        for t in range(NT):
                            for n in range(4):
                                ps = apsum.tile([128, 512], F32, tag="aps")
                                for k in range(4):
                                    nc.tensor.matmul(
                                        ps[:],
                                        enorm[k][:, 128 * t:128 * (t + 1)],
                                        enorm[k][:, 512 * n:512 * (n + 1)],
                                        start=(k == 0), stop=(k == 3))
                                nc.scalar.activation(
                                    a16[t][:, 512 * n:512 * (n + 1)], ps[:],
                                    ACTF.Relu,
                                    accum_out=rowsums[:, 4 * t + n:4 * t + n + 1])
                # enorm freed here; mean: m = (S - B) / B^2
                rs2 = pp.tile([128, 1], F32, tag="rs2")
                nc.vector.tensor_reduce(rs2[:], rowsums[:], axis=AX.X, op=ALU.add)
                sps = psmall.tile([1, 1], F32, tag="ps")
                nc.tensor.matmul(sps[:], ones_col[:], rs2[:], start=True, stop=True)
                m_sb = pp.tile([1, 1], F32, tag="m_sb")
                nc.scalar.activation(m_sb[:], sps[:], ACTF.Copy,
                                     bias=-float(B) / (B * B), scale=1.0 / (B * B))
                mps = psmall.tile([128, 1], F32, tag="ps")
                nc.tensor.matmul(mps[:], ones_row[:], m_sb[:], start=True, stop=True)
                m128 = pp.tile([128, 1], F32, tag="m128")
                nc.scalar.copy(m128[:], mps[:])

                # threshold (in place) + flip (labelled rows/cols = [0, nlab))
                for t in range(NT):
                    eng = nc.vector
                    eng.scalar_tensor_tensor(
                        a16[t][:], a16[t][:], m128[:, 0:1], a16[t][:],
                        op0=ALU.is_ge, op1=ALU.mult)
                    lr = min(max(nlab - 128 * t, 0), 128)  # labelled rows here
                    # flip (x -> 1 - x) everywhere except lab-rows x lab-cols
                    if lr > 0 and nlab < B:
                        nc.scalar.activation(
                            a16[t][:lr, nlab:B], a16[t][:lr, nlab:B],
                            ACTF.Copy, bias=1.0, scale=-1.0)
                    if lr < 128:
                        nc.scalar.activation(
                            a16[t][lr:, :], a16[t][lr:, :],
                            ACTF.Copy, bias=1.0, scale=-1.0)
                    # diagonal: labelled rows -> 0, unlabelled rows -> 1
                    if lr > 0:
                        nc.gpsimd.affine_select(
                            a16[t][:lr, 128 * t:128 * (t + 1)],
                            a16[t][:lr, 128 * t:128 * (t + 1)],
                            pattern=[[1, 128]], compare_op=ALU.not_equal,
                            fill=0.0, base=0, channel_multiplier=-1)
                    if lr < 128:
                        nc.gpsimd.affine_select(
                            a16[t][lr:, 128 * t:128 * (t + 1)],
                            a16[t][lr:, 128 * t:128 * (t + 1)],
                            pattern=[[1, 128]], compare_op=ALU.not_equal,
                            fill=1.0, base=-lr, channel_multiplier=-1)

            # ============= phase 4: GTG iterations =============
            FD = NT * NC_CLS  # 160
            x32 = pp.tile([128, FD], F32, tag="x32")
            nc.sync.dma_start(x32[:], x0_in[:, :])
            xb = pp.tile([128, FD], BF16, tag="xb")
            nc.vector.tensor_copy(xb[:], x32[:])
            active = pp.tile([128, 1], F32, tag="active")
            nc.vector.memset(active[:], 1.0)
            entacc = pp.tile([128, NT], F32, tag="entacc")
            nc.vector.memset(entacc[:], 0.0)

            with tc.tile_pool(name="gtg", bufs=2) as gt, \
                 tc.tile_pool(name="gtg_ps", bufs=2, space="PSUM") as gtps:
                for it in range(maxit):
                    last = (it == maxit - 1)
                    # Y^T = X^T A  (X slices stationary: 10-col weight loads;
                    # A streams at N=512), then PE-transpose back to (128, FD)
                    yt_sb = gt.tile([10, B], F32, tag="yt_sb")
                    for half in range(2):
                        ytp = gtps.tile([10, B // 2], F32, tag=f"yt{half}", bufs=1)
                        for k in range(NT):
                            for n2 in range(2):
                                c0 = 1024 * half + 512 * n2
                                nc.tensor.matmul(
                                    ytp[:, 512 * n2:512 * (n2 + 1)],
                                    xb[:, 10 * k:10 * (k + 1)],
                                    a16[k][:, c0:c0 + 512],
                                    start=(k == 0), stop=(k == NT - 1))
                        nc.scalar.copy(
                            yt_sb[:, 1024 * half:1024 * (half + 1)], ytp[:])
                    yps = gtps.tile([128, FD], F32, tag="yps", bufs=1)
                    for t in range(NT):
                        nc.tensor.transpose(
                            yps[:, 10 * t:10 * (t + 1)],
                            yt_sb[:, 128 * t:128 * (t + 1)],
                            ident[:10, :10])
                    warm = gtps.tile([1, 64], F32, tag="warm", bufs=1)
                    # --- critical chain to X_next (DVE FIFO order matters) ---
                    msb = gt.tile([128, FD], F32, tag="msb")
                    nc.vector.tensor_tensor(msb[:], x32[:], yps[:], op=ALU.mult)
                    r = gt.tile([128, NT], F32, tag="r")
                    nc.vector.tensor_reduce(
                        r[:], msb[:].rearrange("p (t c) -> p t c", c=NC_CLS),
                        axis=AX.X, op=ALU.add)
                    rinv = gt.tile([128, NT], F32, tag="rinv")
                    nc.vector.reciprocal(rinv[:], r[:])
                    xn = gt.tile([128, FD], F32, tag="xn")
                    nc.vector.tensor_tensor(
                        xn[:].rearrange("p (t c) -> p t c", c=NC_CLS),
                        msb[:].rearrange("p (t c) -> p t c", c=NC_CLS),
                        rinv[:, :, None].to_broadcast((128, NT, NC_CLS)),
                        op=ALU.mult)
                    if not last:
                        nc.tensor.matmul(warm[:, 0:1], ones_col[:],
                                         msb[:, 0:1], start=True, stop=True)
                        d = gt.tile([128, FD], F32, tag="d")
                        nc.vector.tensor_tensor(d[:], xn[:], x32[:],
                                                op=ALU.subtract)
                        # X_next = X + active_old * (Xn - X)  (OLD active)
                        nc.vector.scalar_tensor_tensor(
                            x32[:], d[:], active[:, 0:1], x32[:],
                            op0=ALU.mult, op1=ALU.add)
                        nc.vector.tensor_copy(xb[:], x32[:])
                        nc.tensor.matmul(warm[:, 1:2], ones_col[:],
                                         d[:, 0:1], start=True, stop=True)
                    # --- entropy / stop-flag (off the critical path) ---
                    xc = gt.tile([128, FD], F32, tag="xc")
                    nc.vector.tensor_scalar(xc[:], xn[:], 1e-8, 1.0,
                                            op0=ALU.max, op1=ALU.min)
                    lg = gt.tile([128, FD], F32, tag="lg")
                    nc.scalar.activation(lg[:], xc[:], ACTF.Ln)
                    pq = gt.tile([128, FD], F32, tag="pq")
                    nc.gpsimd.tensor_tensor(pq[:], xc[:], lg[:], op=ALU.mult)
                    entsum = gt.tile([128, NT], F32, tag="entsum")
                    nc.vector.tensor_reduce(
                        entsum[:], pq[:].rearrange("p (t c) -> p t c", c=NC_CLS),
                        axis=AX.X, op=ALU.add)
                    nc.vector.scalar_tensor_tensor(
                        entacc[:], entsum[:], active[:, 0:1], entacc[:],
                        op0=ALU.mult, op1=ALU.add)
                    if not last:
                        dsq_s = gt.tile([128, FD], F32, tag="dsq_s")
                        dsq = gt.tile([128, 1], F32, tag="dsq")
                        nc.scalar.activation(dsq_s[:], d[:], ACTF.Square,
                                             accum_out=dsq[:])
                        eps_ = psmall.tile([1, 1], F32, tag="ps")
                        nc.tensor.matmul(eps_[:], ones_col[:], dsq[:],
                                         start=True, stop=True)
                        flag = gt.tile([1, 1], F32, tag="flag")
                        nc.vector.tensor_scalar(flag[:], eps_[:], TOL * TOL,
                                                None, op0=ALU.is_gt)
                        fps = psmall.tile([128, 1], F32, tag="ps")
                        nc.tensor.matmul(fps[:], ones_row[:], flag[:],
                                         start=True, stop=True)
                        # update active for the next iteration (after uses)
                        nc.vector.tensor_tensor(active[:], active[:], fps[:],
                                                op=ALU.mult)

            # q = -entacc / maxit   (layout (p, t): device row = 128 t + p)
            q128 = pp.tile([128, NT], F32, tag="q128")
            nc.vector.tensor_scalar(q128[:], entacc[:], -1.0 / maxit, None,
                                    op0=ALU.mult)
            nc.sync.dma_start(
                q_dram[:].rearrange("(t p) -> p t", p=128), q128[:])
            nc.sync.dma_start(
                q_out[:].rearrange("(t p) -> p t", p=128), q128[:])

            # ============= phase 5: MLP (transposed activations) =============
            def bn_relu(h_ps, P, g_dram, be_dram, out_sb, sp):
                """out = relu(batchnorm(h) * g + be); h_ps: (P, B) psum/sbuf."""
                rowsum = sp.tile([128, 1], F32, tag="bn_rowsum")
                nc.vector.tensor_reduce(rowsum[:P, :], h_ps[:P, :],
                                        axis=AX.X, op=ALU.add)
                sqs = sp.tile([128, B], F32, tag="bn_sqs")
                sq2 = sp.tile([128, 1], F32, tag="bn_sq2")
                nc.scalar.activation(sqs[:P, :], h_ps[:P, :], ACTF.Square,
                                     accum_out=sq2[:P, :])
                mu = sp.tile([128, 1], F32, tag="bn_mu")
                nc.vector.tensor_scalar(mu[:P, :], rowsum[:P, :], 1.0 / B, None,
                                        op0=ALU.mult)
                ex2 = sp.tile([128, 1], F32, tag="bn_ex2")
                nc.vector.tensor_scalar(ex2[:P, :], sq2[:P, :], 1.0 / B, None,
                                        op0=ALU.mult)
                musq = sp.tile([128, 1], F32, tag="bn_musq")
                nc.vector.tensor_tensor(musq[:P, :], mu[:P, :], mu[:P, :],
                                        op=ALU.mult)
                var = sp.tile([128, 1], F32, tag="bn_var")
                nc.vector.tensor_tensor(var[:P, :], ex2[:P, :], musq[:P, :],
                                        op=ALU.subtract)
                nc.vector.tensor_scalar(var[:P, :], var[:P, :], BN_EPS, None,
                                        op0=ALU.add)
                std = sp.tile([128, 1], F32, tag="bn_std")
                nc.scalar.activation(std[:P, :], var[:P, :], ACTF.Sqrt)
                rstd = sp.tile([128, 1], F32, tag="bn_rstd")
                nc.vector.reciprocal(rstd[:P, :], std[:P, :])
                gt_ = sp.tile([128, 1], F32, tag="bn_g")
                nc.sync.dma_start(gt_[:P, :], g_dram[:, :])
                bt_ = sp.tile([128, 1], F32, tag="bn_be")
                nc.sync.dma_start(bt_[:P, :], be_dram[:, :])
                s = sp.tile([128, 1], F32, tag="bn_s")
                nc.vector.tensor_tensor(s[:P, :], gt_[:P, :], rstd[:P, :],
                                        op=ALU.mult)
                nmu = sp.tile([128, 1], F32, tag="bn_nmu")
                nc.vector.tensor_scalar(nmu[:P, :], mu[:P, :], -1.0, None,
                                        op0=ALU.mult)
                bb = sp.tile([128, 1], F32, tag="bn_bb")
                nc.vector.scalar_tensor_tensor(
                    bb[:P, :], nmu[:P, :], s[:P, 0:1], bt_[:P, :],
                    op0=ALU.mult, op1=ALU.add)
                nc.scalar.activation(out_sb[:P, :], h_ps[:P, :], ACTF.Relu,
                                     bias=bb[:P, 0:1], scale=s[:P, 0:1])

            with tc.tile_pool(name="mlp", bufs=1) as mp, \
                 tc.tile_pool(name="mlp_w", bufs=1) as mw, \
                 tc.tile_pool(name="mlp_hps", bufs=1, space="PSUM") as hpool, \
                 tc.tile_pool(name="mlp_xps", bufs=2, space="PSUM") as xpool, \
                 tc.tile_pool(name="mlp_s", bufs=2) as msc:
                er2 = mp.tile([128, B], F32, tag="er2")
                nc.sync.dma_start(er2[:], eraw_dram[:, :])
                qT = mp.tile([1, B], F32, tag="qT")
                nc.sync.dma_start(qT[:], q_dram[:].rearrange("(a b) -> a b", a=1))
                w1a_sb = mw.tile([128, 129], F32, tag="mw1a")
                nc.sync.dma_start(w1a_sb[:], w1a[:, :])
                w1b_sb = mw.tile([1, 129], F32, tag="mw1b")
                nc.sync.dma_start(w1b_sb[:], w1b[:, :])
                # layer 1 main (out rows 0..127)
                h1 = hpool.tile([128, B], F32, tag="hbig")
                for n in range(4):
                    sl = slice(512 * n, 512 * (n + 1))
                    nc.tensor.matmul(h1[:, sl], w1a_sb[:, 0:128], er2[:, sl],
                                     start=True, stop=False)
                    nc.tensor.matmul(h1[:, sl], w1b_sb[:, 0:128], qT[:, sl],
                                     start=False, stop=True)
                z1 = mp.tile([128, B], F32, tag="z1")
                bn_relu(h1, 128, g1a, be1a, z1, msc)
                # layer 1 extra row (out row 128) via chunked psum -> sbuf
                h1x_sb = mp.tile([1, B], F32, tag="h1x_sb")
                for n in range(4):
                    sl = slice(512 * n, 512 * (n + 1))
                    hx = xpool.tile([1, 512], F32, tag="hx")
                    nc.tensor.matmul(hx[:], w1a_sb[:, 128:129], er2[:, sl],
                                     start=True, stop=False)
                    nc.tensor.matmul(hx[:], w1b_sb[:, 128:129], qT[:, sl],
                                     start=False, stop=True)
                    nc.scalar.copy(h1x_sb[:, sl], hx[:])
                z1x = mp.tile([1, B], F32, tag="z1x")
                bn_relu(h1x_sb, 1, g1b, be1b, z1x, msc)
                # layer 2: (129 -> 64)
                w2a_sb = mw.tile([128, 64], F32, tag="mw2a")
                nc.sync.dma_start(w2a_sb[:], w2a[:, :])
                w2b_sb = mw.tile([1, 64], F32, tag="mw2b")
                nc.sync.dma_start(w2b_sb[:], w2b[:, :])
                h2 = hpool.tile([64, B], F32, tag="hbig")
                for n in range(4):
                    sl = slice(512 * n, 512 * (n + 1))
                    nc.tensor.matmul(h2[:, sl], w2a_sb[:, :], z1[:, sl],
                                     start=True, stop=False)
                    nc.tensor.matmul(h2[:, sl], w2b_sb[:, :], z1x[:, sl],
                                     start=False, stop=True)
                z2 = mp.tile([64, B], F32, tag="z2")
                bn_relu(h2, 64, g2, be2, z2, msc)
                # layers 3-5
                zprev, pprev = z2, 64
                for (wd, gd, bed, dout, lname) in [
                        (w3, g3, be3, 64, "3"), (w4, g4, be4, 32, "4"),
                        (w5, g5, be5, 32, "5")]:
                    wsb_ = mw.tile([128, dout], F32, tag=f"wl{lname}")
                    nc.sync.dma_start(wsb_[:pprev, :], wd[:, :])
                    hp = hpool.tile([dout, B], F32, tag="hbig")
                    for n in range(4):
                        sl = slice(512 * n, 512 * (n + 1))
                        nc.tensor.matmul(hp[:, sl], wsb_[:pprev, :],
                                         zprev[:pprev, sl],
                                         start=True, stop=True)
                    znew = mp.tile([dout, B], F32, tag=f"z{lname}")
                    bn_relu(hp, dout, gd, bed, znew, msc)
                    zprev, pprev = znew, dout

                # ===== phase 6: gather branch outputs, final head =====
                nc.sync.dma_start(ag2_in[:, :], zprev[:32, :])
                if solo:
                    nc.sync.dma_start(ag2_out[0, :, :], ag2_in[:, :])
                else:
                    nc.gpsimd.collective_compute(
                        "AllGather", ALU.bypass, replica_groups=rg,
                        ins=[ag2_in[:, :].opt()], outs=[ag2_out[:, :, :].opt()])
                zcat = mp.tile([128, B], F32, tag="zcat")
                nc.sync.dma_start(
                    zcat[:],
                    ag2_out[0:4, :, :].rearrange("r c b -> (r c) b"))
                fw = mw.tile([128, 1], F32, tag="fw")
                nc.sync.dma_start(fw[:], finw[:, :])
                fb = mw.tile([1, 1], F32, tag="fb")
                nc.sync.dma_start(fb[:], finb[:, :])
                yps_sb = mp.tile([1, B], F32, tag="ypsb")
                for n in range(4):
                    sl = slice(512 * n, 512 * (n + 1))
                    yp = xpool.tile([1, 512], F32, tag="hx")
                    nc.tensor.matmul(yp[:], fw[:], zcat[:, sl],
                                     start=True, stop=True)
                    nc.scalar.activation(yps_sb[:, sl], yp[:], ACTF.Identity,
                                         bias=fb[:, 0:1])
                nc.sync.dma_start(
                    ypred_out[:].rearrange("(a b) -> a b", a=1), yps_sb[:])

    nc.finalize()
    return nc


_PROGRAM_CACHE = {}


def _get_program(nlab, maxit=MAXIT):
    key = (nlab, maxit)
    if key not in _PROGRAM_CACHE:
        _PROGRAM_CACHE[key] = _build_program(nlab, maxit)
    return _PROGRAM_CACHE[key]


# --------------------------------------------------------------------------
# host driver
# --------------------------------------------------------------------------

def _softmax(x):
    x = np.asarray(x, np.float32)
    e = np.exp(x - x.max(axis=1, keepdims=True))
    return e / e.sum(axis=1, keepdims=True)


LAST_RESULTS = None


def prepare(features_0, features_1, features_2, features_3, embedds, outs,
            labels, labelled_idx, unlabelled_idx, ls_params, mlp_params,
            final_W, final_b):
    """Host-side input prep: returns (in_maps, perm, nlab_dev, lab)."""
    feats = [np.asarray(f, np.float32) for f in
             (features_0, features_1, features_2, features_3)]
    embedds = np.asarray(embedds, np.float32)
    outs = np.asarray(outs, np.float32)
    lab = np.asarray(labelled_idx).astype(np.int64)
    unlab = np.asarray(unlabelled_idx).astype(np.int64)

    in_lab = np.zeros(B, bool)
    in_lab[lab] = True
    perm = np.concatenate([np.nonzero(in_lab)[0], np.nonzero(~in_lab)[0]])
    nlab_dev = int(in_lab.sum())

    # X0 with the reference's scatter semantics, then permuted + tiled
    probs = _softmax(outs)
    X0 = np.zeros((B, NC_CLS), np.float32)
    X0[lab] = probs[lab]
    X0[unlab] = 1.0 / NC_CLS
    X0p = X0[perm]
    x0_dev = np.ascontiguousarray(
        X0p.reshape(NT, 128, NC_CLS).transpose(1, 0, 2).reshape(128, NT * NC_CLS))

    # embedds: host-normalized, transposed, bf16 (consumed only by core 4)
    en = embedds[perm]
    en = en / np.clip(np.linalg.norm(en, axis=1, keepdims=True), 1e-12, None)
    ebase4 = np.ascontiguousarray(en.T).astype(ml_dtypes.bfloat16).reshape(4, 128, B)
    ebase0 = np.zeros((4, 128, B), ml_dtypes.bfloat16)

    ident = np.eye(128, dtype=np.float32)

    wlist, blist = [], []
    for i in range(4):
        W, bias = ls_params[i]
        wlist.append(np.ascontiguousarray(
            np.asarray(W, np.float32) / (FSS[i] * FSS[i])))
        blist.append(np.ascontiguousarray(
            np.asarray(bias, np.float32).reshape(INTERM, 1)))

    def mlp_pack(br):
        layers = mlp_params[br]
        (W1, _b1, G1, BE1) = [np.asarray(a, np.float32) for a in layers[0]]
        d = {
            "mw1a": np.ascontiguousarray(W1[:128]),
            "mw1b": np.ascontiguousarray(W1[128:129]),
            "g1a": np.ascontiguousarray(G1[:128].reshape(128, 1)),
            "g1b": np.ascontiguousarray(G1[128:129].reshape(1, 1)),
            "be1a": np.ascontiguousarray(BE1[:128].reshape(128, 1)),
            "be1b": np.ascontiguousarray(BE1[128:129].reshape(1, 1)),
        }
        (W2, _b2, G2, BE2) = [np.asarray(a, np.float32) for a in layers[1]]
        d.update(mw2a=np.ascontiguousarray(W2[:128]),
                 mw2b=np.ascontiguousarray(W2[128:129]),
                 g2=np.ascontiguousarray(G2.reshape(-1, 1)),
                 be2=np.ascontiguousarray(BE2.reshape(-1, 1)))
        for li, nm in [(2, "3"), (3, "4"), (4, "5")]:
            (Wl, _bl, Gl, BEl) = [np.asarray(a, np.float32) for a in layers[li]]
            d["mw" + nm] = np.ascontiguousarray(Wl)
            d["g" + nm] = np.ascontiguousarray(Gl.reshape(-1, 1))
            d["be" + nm] = np.ascontiguousarray(BEl.reshape(-1, 1))
        return d

    finw_np = np.ascontiguousarray(np.asarray(final_W, np.float32).reshape(128, 1))
    finb_np = np.ascontiguousarray(np.asarray(final_b, np.float32).reshape(1, 1))

    featp = [np.take(f.reshape(B, -1), perm, axis=0) for f in feats]

    in_maps = []
    for c in range(NCORES):
        m = {}
        for i in range(4):
            m[f"f{i}"] = featp[i][BLOC * c:BLOC * (c + 1)]
            m[f"w{i}"] = wlist[i]
            m[f"b{i}"] = blist[i]
        m["ebase"] = ebase4 if c == 4 else ebase0
        am = np.zeros((128, 5), np.float32)
        if c < 4:
            am[:, c] = 1.0
        elif c == 4:
            am[:, 4] = 1.0
        m["amask"] = am
        m["x0"] = x0_dev
        m["ident"] = ident
        m.update(mlp_pack(c if c < 4 else 0))
        m["finw"] = finw_np
        m["finb"] = finb_np
        in_maps.append(m)

    return in_maps, perm, nlab_dev, lab


def kernel(features_0, features_1, features_2, features_3, embedds, outs,
           labels, labelled_idx, unlabelled_idx, ls_params, mlp_params,
           final_W, final_b, _maxit=MAXIT, _trace=False):
    in_maps, perm, nlab_dev, lab = prepare(
        features_0, features_1, features_2, features_3, embedds, outs,
        labels, labelled_idx, unlabelled_idx, ls_params, mlp_params,
        final_W, final_b)
    nc = _get_program(nlab_dev, _maxit)
    res = run_bass_kernel_spmd(nc, in_maps, core_ids=list(range(NCORES)),
                               trace=_trace)
    global LAST_RESULTS
    LAST_RESULTS = res
    outs_list = res.results

    inv = np.empty(B, np.int64)
    inv[perm] = np.arange(B)
    y_pred = np.asarray(outs_list[0]["y_pred"], np.float32)[inv]
    y_true = np.asarray(outs_list[4]["q_out"], np.float32)[inv]
    mask = np.zeros(B, np.float32)
    mask[lab] = 1.0
    return y_pred, y_true, mask.astype(bool)


# revision 20
# speedup vs baseline: 1.0379x; 1.0379x over previous
"""Trainium2 Bass kernel for nn_GTGModule (GTG message passing + MLP heads).

Self-contained: accepts FULL inputs (as produced by the problem's
setup_inputs), shards across 8 NeuronCores internally, returns FULL outputs
(y_pred, y_true, mask).

Strategy (single SPMD launch, 8 cores, 3 AllGathers):
  - batch-sharded feature pooling (the ~1GB memory-bound part)
  - host pre-permutes the batch so labelled rows are [0, nlab) => the
    labelled x labelled block of the affinity matrix is tile-contiguous
  - each of cores 0-3 runs one branch's full B x B GTG; core 4 runs the
    embedds GTG; selection is done with per-core 0/1 mask inputs (pure SPMD)
  - MLP runs with transposed activations so BatchNorm is a free-dim reduce
"""

import numpy as np
import ml_dtypes

import concourse.bass as bass
import concourse.bacc as bacc
import concourse.bass_isa as bass_isa
import concourse.mybir as mybir
from concourse import tile
from concourse.bass_utils import run_bass_kernel_spmd

F32 = mybir.dt.float32
BF16 = mybir.dt.bfloat16
ALU = mybir.AluOpType
ACTF = mybir.ActivationFunctionType
AX = mybir.AxisListType

B = 2048
NC_CLS = 10
NCORES = 8
BLOC = B // NCORES          # 256 batch rows per core
CHS = [64, 128, 256, 512]
FSS = [32, 16, 8, 4]
INTERM = 128
MAXIT = 30
TOL = 1e-3
BN_EPS = 1e-5
NT = B // 128               # 16 row blocks
DMA_CHUNK = 4096            # free elems per pooling DMA tile (16KB/partition)


# --------------------------------------------------------------------------
# device program
# --------------------------------------------------------------------------

def _build_program(nlab, maxit=MAXIT, solo=False):
    nc = bacc.Bacc("TRN2", num_devices=NCORES)

    # ---------------- inputs ----------------
    feats = [nc.dram_tensor(f"f{i}", [BLOC, CHS[i] * FSS[i] * FSS[i]], F32,
                            kind="ExternalInput") for i in range(4)]
    ws = [nc.dram_tensor(f"w{i}", [CHS[i], INTERM], F32, kind="ExternalInput")
          for i in range(4)]
    bs = [nc.dram_tensor(f"b{i}", [INTERM, 1], F32, kind="ExternalInput")
          for i in range(4)]
    ebase = nc.dram_tensor("ebase", [4, 128, B], BF16, kind="ExternalInput")
    amask_in = nc.dram_tensor("amask", [128, 5], F32, kind="ExternalInput")
    x0_in = nc.dram_tensor("x0", [128, NT * NC_CLS], F32, kind="ExternalInput")
    ident_in = nc.dram_tensor("ident", [128, 128], F32, kind="ExternalInput")
    w1a = nc.dram_tensor("mw1a", [128, 129], F32, kind="ExternalInput")
    w1b = nc.dram_tensor("mw1b", [1, 129], F32, kind="ExternalInput")
    w2a = nc.dram_tensor("mw2a", [128, 64], F32, kind="ExternalInput")
    w2b = nc.dram_tensor("mw2b", [1, 64], F32, kind="ExternalInput")
    w3 = nc.dram_tensor("mw3", [64, 64], F32, kind="ExternalInput")
    w4 = nc.dram_tensor("mw4", [64, 32], F32, kind="ExternalInput")
    w5 = nc.dram_tensor("mw5", [32, 32], F32, kind="ExternalInput")
    g1a = nc.dram_tensor("g1a", [128, 1], F32, kind="ExternalInput")
    g1b = nc.dram_tensor("g1b", [1, 1], F32, kind="ExternalInput")
    be1a = nc.dram_tensor("be1a", [128, 1], F32, kind="ExternalInput")
    be1b = nc.dram_tensor("be1b", [1, 1], F32, kind="ExternalInput")
    g2 = nc.dram_tensor("g2", [64, 1], F32, kind="ExternalInput")
    be2 = nc.dram_tensor("be2", [64, 1], F32, kind="ExternalInput")
    g3 = nc.dram_tensor("g3", [64, 1], F32, kind="ExternalInput")
    be3 = nc.dram_tensor("be3", [64, 1], F32, kind="ExternalInput")
    g4 = nc.dram_tensor("g4", [32, 1], F32, kind="ExternalInput")
    be4 = nc.dram_tensor("be4", [32, 1], F32, kind="ExternalInput")
    g5 = nc.dram_tensor("g5", [32, 1], F32, kind="ExternalInput")
    be5 = nc.dram_tensor("be5", [32, 1], F32, kind="ExternalInput")
    finw = nc.dram_tensor("finw", [128, 1], F32, kind="ExternalInput")
    finb = nc.dram_tensor("finb", [1, 1], F32, kind="ExternalInput")

    # ---------------- outputs ----------------
    ypred_out = nc.dram_tensor("y_pred", [B], F32, kind="ExternalOutput")
    q_out = nc.dram_tensor("q_out", [B], F32, kind="ExternalOutput")

    # ---------------- internal DRAM ----------------
    ag1n_in = nc.dram_tensor("ag1n_in", [4, 128, BLOC], BF16, kind="Internal")
    ag1n_out = nc.dram_tensor("ag1n_out", [NCORES, 4, 128, BLOC], BF16,
                              kind="Internal", addr_space="Shared")
    ag1r_in = nc.dram_tensor("ag1r_in", [4, 128, BLOC], F32, kind="Internal")
    ag1r_out = nc.dram_tensor("ag1r_out", [NCORES, 4, 128, BLOC], F32,
                              kind="Internal", addr_space="Shared")
    ag2_in = nc.dram_tensor("ag2_in", [32, B], F32, kind="Internal")
    ag2_out = nc.dram_tensor("ag2_out", [NCORES, 32, B], F32,
                             kind="Internal", addr_space="Shared")
    eraw_dram = nc.dram_tensor("eraw_dram", [128, B], F32, kind="Internal")
    q_dram = nc.dram_tensor("q_dram", [B], F32, kind="Internal")
    rg = [list(range(NCORES))]

    with tile.TileContext(nc) as tc:
        with tc.tile_pool(name="persist", bufs=1) as pp, \
             tc.tile_pool(name="a16", bufs=1) as a16pool, \
             tc.tile_pool(name="psmall", bufs=2, space="PSUM") as psmall:

            ident = pp.tile([128, 128], F32, tag="ident")
            nc.sync.dma_start(ident[:], ident_in[:, :])
            amask = pp.tile([128, 5], F32, tag="amask")
            nc.sync.dma_start(amask[:], amask_in[:, :])
            ones_col = pp.tile([128, 1], F32, tag="ones_col")
            nc.vector.memset(ones_col[:], 1.0)
            ones_row = pp.tile([1, 128], F32, tag="ones_row")
            nc.vector.memset(ones_row[:], 1.0)
            ones10 = pp.tile([10, 1], F32, tag="ones10")
            nc.vector.memset(ones10[:], 1.0)

            # ============= phase 1: pooling + branch embeddings =============
            with tc.tile_pool(name="pool_dma", bufs=4) as fpool, \
                 tc.tile_pool(name="pool_rm", bufs=2) as prpool, \
                 tc.tile_pool(name="pool_w", bufs=1) as wpool, \
                 tc.tile_pool(name="pooledT", bufs=1) as ptpool, \
                 tc.tile_pool(name="emb", bufs=1) as embpool, \
                 tc.tile_pool(name="pool_ps", bufs=2, space="PSUM") as ppsum, \
                 tc.tile_pool(name="emb_ps", bufs=2, space="PSUM") as epsum:
                embraw = []   # (128, BLOC) f32, per branch
                ntpart = []   # (128, BLOC) bf16, per branch (row-normalized)
                for i in range(4):
                    CH, S = CHS[i], FSS[i] * FSS[i]
                    row = CH * S
                    nchunks = row // DMA_CHUNK
                    chpc = DMA_CHUNK // S          # channels per chunk
                    nkchunk = (CH + 127) // 128    # partition chunks of pooledT
                    ptiles = [ptpool.tile([128, BLOC], F32, tag=f"pt{i}_{j}",
                                          name=f"pt{i}_{j}")
                              for j in range(nkchunk)]
                    for bt in range(BLOC // 128):
                        pr = prpool.tile([128, CH], F32, tag="poolrm")
                        for c in range(nchunks):
                            ft = fpool.tile([128, DMA_CHUNK], F32, tag="ftile")
                            nc.sync.dma_start(
                                ft[:],
                                feats[i][128 * bt:128 * (bt + 1),
                                         DMA_CHUNK * c:DMA_CHUNK * (c + 1)])
                            nc.vector.tensor_reduce(
                                pr[:, chpc * c:chpc * (c + 1)],
                                ft[:].rearrange("p (ch s) -> p ch s", s=S),
                                axis=AX.X, op=ALU.add)
                        # transpose (128, CH) -> (CH, 128) into pooledT chunks
                        for j in range(nkchunk):
                            w = min(128, CH - 128 * j)
                            tp = ppsum.tile([128, 128], F32, tag="tpsum")
                            nc.tensor.transpose(
                                tp[:w, :], pr[:, 128 * j:128 * j + w], ident[:])
                            nc.scalar.copy(
                                ptiles[j][:w, 128 * bt:128 * (bt + 1)], tp[:w, :])
                    # emb = relu(W.T @ pooledT + bias): out (128, BLOC)
                    kp = min(128, CH)
                    wsb = wpool.tile([128, nkchunk * INTERM], F32, tag=f"wsb{i}")
                    nc.sync.dma_start(
                        wsb[:kp, :].rearrange("p (k m) -> p k m", m=INTERM),
                        ws[i][:, :].rearrange("(k p) m -> p k m", p=kp))
                    bsb = wpool.tile([128, 1], F32, tag=f"bsb{i}")
                    nc.sync.dma_start(bsb[:], bs[i][:, :])
                    eps = epsum.tile([128, BLOC], F32, tag="embps")
                    for j in range(nkchunk):
                        w = min(128, CH - 128 * j)
                        nc.tensor.matmul(
                            eps[:, :], wsb[:w, INTERM * j:INTERM * (j + 1)],
                            ptiles[j][:w, :],
                            start=(j == 0), stop=(j == nkchunk - 1))
                    er = embpool.tile([128, BLOC], F32, tag=f"embraw{i}")
                    nc.scalar.activation(er[:], eps[:], ACTF.Relu, bias=bsb[:, 0:1])
                    embraw.append(er)
                    # row-normalize (norm over the 128 channels = partition dim)
                    sq = prpool.tile([128, BLOC], F32, tag="sqscratch")
                    nc.scalar.activation(sq[:], er[:], ACTF.Square)
                    n2 = psmall.tile([1, BLOC], F32, tag="ps")
                    nc.tensor.matmul(n2[:], ones_col[:], sq[:], start=True, stop=True)
                    nrm = prpool.tile([1, BLOC], F32, tag="nrm")
                    nc.scalar.activation(nrm[:], n2[:], ACTF.Sqrt)
                    nc.vector.tensor_scalar(nrm[:], nrm[:], 1e-12, None, op0=ALU.max)
                    rn = prpool.tile([1, BLOC], F32, tag="rn")
                    nc.vector.reciprocal(rn[:], nrm[:])
                    rnb = psmall.tile([128, BLOC], F32, tag="ps")
                    nc.tensor.matmul(rnb[:], ones_row[:], rn[:], start=True, stop=True)
                    nt_ = embpool.tile([128, BLOC], BF16, tag=f"ntpart{i}")
                    nc.vector.tensor_tensor(nt_[:], er[:], rnb[:], op=ALU.mult)
                    ntpart.append(nt_)

                # ===== phase 2: AllGather emb parts =====
                for i in range(4):
                    nc.sync.dma_start(ag1n_in[i, :, :], ntpart[i][:])
                    nc.sync.dma_start(ag1r_in[i, :, :], embraw[i][:])
            if solo:
                nc.sync.dma_start(ag1n_out[0, :, :, :], ag1n_in[:, :, :])
                nc.sync.dma_start(ag1r_out[0, :, :, :], ag1r_in[:, :, :])
            else:
                nc.gpsimd.collective_compute(
                    "AllGather", ALU.bypass, replica_groups=rg,
                    ins=[ag1n_in[:, :, :].opt()],
                    outs=[ag1n_out[:, :, :, :].opt()])
                nc.gpsimd.collective_compute(
                    "AllGather", ALU.bypass, replica_groups=rg,
                    ins=[ag1r_in[:, :, :].opt()],
                    outs=[ag1r_out[:, :, :, :].opt()])

            # ============= phase 3: blend per-core E, build A =============
            a16 = [a16pool.tile([128, B], BF16, tag=f"a16_{t}", name=f"a16_{t}")
                   for t in range(NT)]
            rowsums = pp.tile([128, 4 * NT], F32, tag="rowsums")
            if True:
                with tc.tile_pool(name="enorm", bufs=1) as enpool:
                    enorm = [enpool.tile([128, B], BF16, tag=f"en{q}", name=f"en{q}")
                             for q in range(4)]
                    with tc.tile_pool(name="blend", bufs=2) as blpool:
                        # raw E blend (chunk 0 only; branch cores only)
                        eraw = blpool.tile([128, B], F32, tag="erawblend")
                        for i in range(4):
                            rb = blpool.tile([128, B], F32, tag="ldtile")
                            nc.sync.dma_start(
                                rb[:].rearrange("p (r b) -> p r b", b=BLOC),
                                ag1r_out[:, i, :, :].rearrange("r p b -> p r b"))
                            if i == 0:
                                nc.vector.tensor_scalar(
                                    eraw[:], rb[:], amask[:, 0:1], None,
                                    op0=ALU.mult)
                            else:
                                nc.vector.scalar_tensor_tensor(
                                    eraw[:], rb[:], amask[:, i:i + 1], eraw[:],
                                    op0=ALU.mult, op1=ALU.add)
                        nc.sync.dma_start(eraw_dram[:, :], eraw[:])
                        # normalized E blend: chunk 0 from branches + ebase
                        for i in range(4):
                            nb = blpool.tile([128, B], BF16, tag="ldtile")
                            nc.sync.dma_start(
                                nb[:].rearrange("p (r b) -> p r b", b=BLOC),
                                ag1n_out[:, i, :, :].rearrange("r p b -> p r b"))
                            if i == 0:
                                nc.vector.tensor_scalar(
                                    enorm[0][:], nb[:], amask[:, 0:1], None,
                                    op0=ALU.mult)
                            else:
                                nc.vector.scalar_tensor_tensor(
                                    enorm[0][:], nb[:], amask[:, i:i + 1],
                                    enorm[0][:], op0=ALU.mult, op1=ALU.add)
                        eb = blpool.tile([128, B], BF16, tag="ldtile")
                        nc.sync.dma_start(eb[:], ebase[0, :, :])
                        nc.vector.scalar_tensor_tensor(
                            enorm[0][:], eb[:], amask[:, 4:5], enorm[0][:],
                            op0=ALU.mult, op1=ALU.add)
                        for qch in range(1, 4):
                            ebq = blpool.tile([128, B], BF16, tag="ldtile")
                            nc.sync.dma_start(ebq[:], ebase[qch, :, :])
                            nc.vector.tensor_scalar(
                                enorm[qch][:], ebq[:], amask[:, 4:5], None,
                                op0=ALU.mult)

                    # A = relu(E^T E) with fused fp32 row-sums, stored bf16
                    with tc.tile_pool(name="aps", bufs=4, space="PSUM") as apsum:
                        for t in range(NT):
                            for n in range(4):
                                ps = apsum.tile([128, 512], F32, tag="aps")
                                for k in range(4):
                                    nc.tensor.matmul(
                                        ps[:],
                                        enorm[k][:, 128 * t:128 * (t + 1)],
                                        enorm[k][:, 512 * n:512 * (n + 1)],
                                        start=(k == 0), stop=(k == 3))
                                nc.scalar.activation(
                                    a16[t][:, 512 * n:512 * (n + 1)], ps[:],
                                    ACTF.Relu,
                                    accum_out=rowsums[:, 4 * t + n:4 * t + n + 1])
                # enorm freed here; mean: m = (S - B) / B^2
                rs2 = pp.tile([128, 1], F32, tag="rs2")
                nc.vector.tensor_reduce(rs2[:], rowsums[:], axis=AX.X, op=ALU.add)
                sps = psmall.tile([1, 1], F32, tag="ps")
                nc.tensor.matmul(sps[:], ones_col[:], rs2[:], start=True, stop=True)
                m_sb = pp.tile([1, 1], F32, tag="m_sb")
                nc.scalar.activation(m_sb[:], sps[:], ACTF.Copy,
                                     bias=-float(B) / (B * B), scale=1.0 / (B * B))
                mps = psmall.tile([128, 1], F32, tag="ps")
                nc.tensor.matmul(mps[:], ones_row[:], m_sb[:], start=True, stop=True)
                m128 = pp.tile([128, 1], F32, tag="m128")
                nc.scalar.copy(m128[:], mps[:])

                # threshold (in place) + flip (labelled rows/cols = [0, nlab))
                for t in range(NT):
                    eng = nc.vector
                    eng.scalar_tensor_tensor(
                        a16[t][:], a16[t][:], m128[:, 0:1], a16[t][:],
                        op0=ALU.is_ge, op1=ALU.mult)
                    lr = min(max(nlab - 128 * t, 0), 128)  # labelled rows here
                    # flip (x -> 1 - x) everywhere except lab-rows x lab-cols
                    if lr > 0 and nlab < B:
                        nc.scalar.activation(
                            a16[t][:lr, nlab:B], a16[t][:lr, nlab:B],
                            ACTF.Copy, bias=1.0, scale=-1.0)
                    if lr < 128:
                        nc.scalar.activation(
                            a16[t][lr:, :], a16[t][lr:, :],
                            ACTF.Copy, bias=1.0, scale=-1.0)
                    # diagonal: labelled rows -> 0, unlabelled rows -> 1
                    if lr > 0:
                        nc.gpsimd.affine_select(
                            a16[t][:lr, 128 * t:128 * (t + 1)],
                            a16[t][:lr, 128 * t:128 * (t + 1)],
                            pattern=[[1, 128]], compare_op=ALU.not_equal,
                            fill=0.0, base=0, channel_multiplier=-1)
                    if lr < 128:
                        nc.gpsimd.affine_select(
                            a16[t][lr:, 128 * t:128 * (t + 1)],
                            a16[t][lr:, 128 * t:128 * (t + 1)],
                            pattern=[[1, 128]], compare_op=ALU.not_equal,
                            fill=1.0, base=-lr, channel_multiplier=-1)

            # ============= phase 4: GTG iterations =============
            FD = NT * NC_CLS  # 160
            x32 = pp.tile([128, FD], F32, tag="x32")
            nc.sync.dma_start(x32[:], x0_in[:, :])
            xb = pp.tile([128, FD], BF16, tag="xb")
            nc.vector.tensor_copy(xb[:], x32[:])
            active = pp.tile([128, 1], F32, tag="active")
            nc.vector.memset(active[:], 1.0)
            entacc = pp.tile([128, NT], F32, tag="entacc")
            nc.vector.memset(entacc[:], 0.0)

            with tc.tile_pool(name="gtg", bufs=2) as gt, \
                 tc.tile_pool(name="gtg_ps", bufs=2, space="PSUM") as gtps:
                for it in range(maxit):
                    last = (it == maxit - 1)
                    # Y^T = X^T A  (X slices stationary: 10-col weight loads;
                    # A streams at N=512), then PE-transpose back to (128, FD)
                    yt_sb = gt.tile([10, B], F32, tag="yt_sb")
                    for half in range(2):
                        ytp = gtps.tile([10, B // 2], F32, tag=f"yt{half}", bufs=1)
                        for k in range(NT):
                            for n2 in range(2):
                                c0 = 1024 * half + 512 * n2
                                nc.tensor.matmul(
                                    ytp[:, 512 * n2:512 * (n2 + 1)],
                                    xb[:, 10 * k:10 * (k + 1)],
                                    a16[k][:, c0:c0 + 512],
                                    start=(k == 0), stop=(k == NT - 1))
                        nc.scalar.copy(
                            yt_sb[:, 1024 * half:1024 * (half + 1)], ytp[:])
                    yps = gtps.tile([128, FD], F32, tag="yps", bufs=1)
                    for t in range(NT):
                        nc.tensor.transpose(
                            yps[:, 10 * t:10 * (t + 1)],
                            yt_sb[:, 128 * t:128 * (t + 1)],
                            ident[:10, :10])
                    warm = gtps.tile([1, 64], F32, tag="warm", bufs=1)
                    # --- critical chain to X_next (DVE FIFO order matters) ---
                    msb = gt.tile([128, FD], F32, tag="msb")
                    nc.vector.tensor_tensor(msb[:], x32[:], yps[:], op=ALU.mult)
                    r = gt.tile([128, NT], F32, tag="r")
                    nc.vector.tensor_reduce(
                        r[:], msb[:].rearrange("p (t c) -> p t c", c=NC_CLS),
                        axis=AX.X, op=ALU.add)
                    rinv = gt.tile([128, NT], F32, tag="rinv")
                    nc.vector.reciprocal(rinv[:], r[:])
                    xn = gt.tile([128, FD], F32, tag="xn")
                    nc.vector.tensor_tensor(
                        xn[:].rearrange("p (t c) -> p t c", c=NC_CLS),
                        msb[:].rearrange("p (t c) -> p t c", c=NC_CLS),
                        rinv[:, :, None].to_broadcast((128, NT, NC_CLS)),
                        op=ALU.mult)
                    if not last:
                        nc.tensor.matmul(warm[:, 0:1], ones_col[:],
                                         msb[:, 0:1], start=True, stop=True)
                        d = gt.tile([128, FD], F32, tag="d")
                        nc.vector.tensor_tensor(d[:], xn[:], x32[:],
                                                op=ALU.subtract)
                        # X_next = X + active_old * (Xn - X)  (OLD active)
                        nc.vector.scalar_tensor_tensor(
                            x32[:], d[:], active[:, 0:1], x32[:],
                            op0=ALU.mult, op1=ALU.add)
                        nc.vector.tensor_copy(xb[:], x32[:])
                        nc.tensor.matmul(warm[:, 1:2], ones_col[:],
                                         d[:, 0:1], start=True, stop=True)
                    # --- entropy / stop-flag (off the critical path) ---
                    xc = gt.tile([128, FD], F32, tag="xc")
                    nc.vector.tensor_scalar(xc[:], xn[:], 1e-8, 1.0,
                                            op0=ALU.max, op1=ALU.min)
                    lg = gt.tile([128, FD], F32, tag="lg")
                    nc.scalar.activation(lg[:], xc[:], ACTF.Ln)
                    pq = gt.tile([128, FD], F32, tag="pq")
                    nc.gpsimd.tensor_tensor(pq[:], xc[:], lg[:], op=ALU.mult)
                    entsum = gt.tile([128, NT], F32, tag="entsum")
                    nc.vector.tensor_reduce(
                        entsum[:], pq[:].rearrange("p (t c) -> p t c", c=NC_CLS),
                        axis=AX.X, op=ALU.add)
                    nc.vector.scalar_tensor_tensor(
                        entacc[:], entsum[:], active[:, 0:1], entacc[:],
                        op0=ALU.mult, op1=ALU.add)
                    if not last:
                        dsq_s = gt.tile([128, FD], F32, tag="dsq_s")
                        dsq = gt.tile([128, 1], F32, tag="dsq")
                        nc.scalar.activation(dsq_s[:], d[:], ACTF.Square,
                                             accum_out=dsq[:])
                        # cross-partition err reduction on GPSIMD: keeps the
                        # PE FIFO free of late-dependency matmuls
                        errall = gt.tile([128, 1], F32, tag="errall")
                        nc.gpsimd.partition_all_reduce(
                            errall[:], dsq[:], channels=128,
                            reduce_op=bass_isa.ReduceOp.add)
                        flag = gt.tile([128, 1], F32, tag="flag")
                        nc.vector.tensor_scalar(flag[:], errall[:], TOL * TOL,
                                                None, op0=ALU.is_gt)
                        # update active for the next iteration (after uses)
                        nc.vector.tensor_tensor(active[:], active[:], flag[:],
                                                op=ALU.mult)

            # q = -entacc / maxit   (layout (p, t): device row = 128 t + p)
            q128 = pp.tile([128, NT], F32, tag="q128")
            nc.vector.tensor_scalar(q128[:], entacc[:], -1.0 / maxit, None,
                                    op0=ALU.mult)
            nc.sync.dma_start(
                q_dram[:].rearrange("(t p) -> p t", p=128), q128[:])
            nc.sync.dma_start(
                q_out[:].rearrange("(t p) -> p t", p=128), q128[:])

            # ============= phase 5: MLP (transposed activations) =============
            def bn_relu(h_ps, P, g_dram, be_dram, out_sb, sp):
                """out = relu(batchnorm(h) * g + be); h_ps: (P, B) psum/sbuf."""
                rowsum = sp.tile([128, 1], F32, tag="bn_rowsum")
                nc.vector.tensor_reduce(rowsum[:P, :], h_ps[:P, :],
                                        axis=AX.X, op=ALU.add)
                sqs = sp.tile([128, B], F32, tag="bn_sqs")
                sq2 = sp.tile([128, 1], F32, tag="bn_sq2")
                nc.scalar.activation(sqs[:P, :], h_ps[:P, :], ACTF.Square,
                                     accum_out=sq2[:P, :])
                mu = sp.tile([128, 1], F32, tag="bn_mu")
                nc.vector.tensor_scalar(mu[:P, :], rowsum[:P, :], 1.0 / B, None,
                                        op0=ALU.mult)
                ex2 = sp.tile([128, 1], F32, tag="bn_ex2")
                nc.vector.tensor_scalar(ex2[:P, :], sq2[:P, :], 1.0 / B, None,
                                        op0=ALU.mult)
                musq = sp.tile([128, 1], F32, tag="bn_musq")
                nc.vector.tensor_tensor(musq[:P, :], mu[:P, :], mu[:P, :],
                                        op=ALU.mult)
                var = sp.tile([128, 1], F32, tag="bn_var")
                nc.vector.tensor_tensor(var[:P, :], ex2[:P, :], musq[:P, :],
                                        op=ALU.subtract)
                nc.vector.tensor_scalar(var[:P, :], var[:P, :], BN_EPS, None,
                                        op0=ALU.add)
                std = sp.tile([128, 1], F32, tag="bn_std")
                nc.scalar.activation(std[:P, :], var[:P, :], ACTF.Sqrt)
                rstd = sp.tile([128, 1], F32, tag="bn_rstd")
                nc.vector.reciprocal(rstd[:P, :], std[:P, :])
                gt_ = sp.tile([128, 1], F32, tag="bn_g")
                nc.sync.dma_start(gt_[:P, :], g_dram[:, :])
                bt_ = sp.tile([128, 1], F32, tag="bn_be")
                nc.sync.dma_start(bt_[:P, :], be_dram[:, :])
                s = sp.tile([128, 1], F32, tag="bn_s")
                nc.vector.tensor_tensor(s[:P, :], gt_[:P, :], rstd[:P, :],
                                        op=ALU.mult)
                nmu = sp.tile([128, 1], F32, tag="bn_nmu")
                nc.vector.tensor_scalar(nmu[:P, :], mu[:P, :], -1.0, None,
                                        op0=ALU.mult)
                bb = sp.tile([128, 1], F32, tag="bn_bb")
                nc.vector.scalar_tensor_tensor(
                    bb[:P, :], nmu[:P, :], s[:P, 0:1], bt_[:P, :],
                    op0=ALU.mult, op1=ALU.add)
                nc.scalar.activation(out_sb[:P, :], h_ps[:P, :], ACTF.Relu,
                                     bias=bb[:P, 0:1], scale=s[:P, 0:1])

            with tc.tile_pool(name="mlp", bufs=1) as mp, \
                 tc.tile_pool(name="mlp_w", bufs=1) as mw, \
                 tc.tile_pool(name="mlp_hps", bufs=1, space="PSUM") as hpool, \
                 tc.tile_pool(name="mlp_xps", bufs=2, space="PSUM") as xpool, \
                 tc.tile_pool(name="mlp_s", bufs=2) as msc:
                er2 = mp.tile([128, B], F32, tag="er2")
                nc.sync.dma_start(er2[:], eraw_dram[:, :])
                qT = mp.tile([1, B], F32, tag="qT")
                nc.sync.dma_start(qT[:], q_dram[:].rearrange("(a b) -> a b", a=1))
                w1a_sb = mw.tile([128, 129], F32, tag="mw1a")
                nc.sync.dma_start(w1a_sb[:], w1a[:, :])
                w1b_sb = mw.tile([1, 129], F32, tag="mw1b")
                nc.sync.dma_start(w1b_sb[:], w1b[:, :])
                # layer 1 main (out rows 0..127)
                h1 = hpool.tile([128, B], F32, tag="hbig")
                for n in range(4):
                    sl = slice(512 * n, 512 * (n + 1))
                    nc.tensor.matmul(h1[:, sl], w1a_sb[:, 0:128], er2[:, sl],
                                     start=True, stop=False)
                    nc.tensor.matmul(h1[:, sl], w1b_sb[:, 0:128], qT[:, sl],
                                     start=False, stop=True)
                z1 = mp.tile([128, B], F32, tag="z1")
                bn_relu(h1, 128, g1a, be1a, z1, msc)
                # layer 1 extra row (out row 128) via chunked psum -> sbuf
                h1x_sb = mp.tile([1, B], F32, tag="h1x_sb")
                for n in range(4):
                    sl = slice(512 * n, 512 * (n + 1))
                    hx = xpool.tile([1, 512], F32, tag="hx")
                    nc.tensor.matmul(hx[:], w1a_sb[:, 128:129], er2[:, sl],
                                     start=True, stop=False)
                    nc.tensor.matmul(hx[:], w1b_sb[:, 128:129], qT[:, sl],
                                     start=False, stop=True)
                    nc.scalar.copy(h1x_sb[:, sl], hx[:])
                z1x = mp.tile([1, B], F32, tag="z1x")
                bn_relu(h1x_sb, 1, g1b, be1b, z1x, msc)
                # layer 2: (129 -> 64)
                w2a_sb = mw.tile([128, 64], F32, tag="mw2a")
                nc.sync.dma_start(w2a_sb[:], w2a[:, :])
                w2b_sb = mw.tile([1, 64], F32, tag="mw2b")
                nc.sync.dma_start(w2b_sb[:], w2b[:, :])
                h2 = hpool.tile([64, B], F32, tag="hbig")
                for n in range(4):
                    sl = slice(512 * n, 512 * (n + 1))
                    nc.tensor.matmul(h2[:, sl], w2a_sb[:, :], z1[:, sl],
                                     start=True, stop=False)
                    nc.tensor.matmul(h2[:, sl], w2b_sb[:, :], z1x[:, sl],
                                     start=False, stop=True)
                z2 = mp.tile([64, B], F32, tag="z2")
                bn_relu(h2, 64, g2, be2, z2, msc)
                # layers 3-5
                zprev, pprev = z2, 64
                for (wd, gd, bed, dout, lname) in [
                        (w3, g3, be3, 64, "3"), (w4, g4, be4, 32, "4"),
                        (w5, g5, be5, 32, "5")]:
                    wsb_ = mw.tile([128, dout], F32, tag=f"wl{lname}")
                    nc.sync.dma_start(wsb_[:pprev, :], wd[:, :])
                    hp = hpool.tile([dout, B], F32, tag="hbig")
                    for n in range(4):
                        sl = slice(512 * n, 512 * (n + 1))
                        nc.tensor.matmul(hp[:, sl], wsb_[:pprev, :],
                                         zprev[:pprev, sl],
                                         start=True, stop=True)
                    znew = mp.tile([dout, B], F32, tag=f"z{lname}")
                    bn_relu(hp, dout, gd, bed, znew, msc)
                    zprev, pprev = znew, dout

                # ===== phase 6: gather branch outputs, final head =====
                nc.sync.dma_start(ag2_in[:, :], zprev[:32, :])
                if solo:
                    nc.sync.dma_start(ag2_out[0, :, :], ag2_in[:, :])
                else:
                    nc.gpsimd.collective_compute(
                        "AllGather", ALU.bypass, replica_groups=rg,
                        ins=[ag2_in[:, :].opt()], outs=[ag2_out[:, :, :].opt()])
                zcat = mp.tile([128, B], F32, tag="zcat")
                nc.sync.dma_start(
                    zcat[:],
                    ag2_out[0:4, :, :].rearrange("r c b -> (r c) b"))
                fw = mw.tile([128, 1], F32, tag="fw")
                nc.sync.dma_start(fw[:], finw[:, :])
                fb = mw.tile([1, 1], F32, tag="fb")
                nc.sync.dma_start(fb[:], finb[:, :])
                yps_sb = mp.tile([1, B], F32, tag="ypsb")
                for n in range(4):
                    sl = slice(512 * n, 512 * (n + 1))
                    yp = xpool.tile([1, 512], F32, tag="hx")
                    nc.tensor.matmul(yp[:], fw[:], zcat[:, sl],
                                     start=True, stop=True)
                    nc.scalar.activation(yps_sb[:, sl], yp[:], ACTF.Identity,
                                         bias=fb[:, 0:1])
                nc.sync.dma_start(
                    ypred_out[:].rearrange("(a b) -> a b", a=1), yps_sb[:])

    nc.finalize()
    return nc


_PROGRAM_CACHE = {}


def _get_program(nlab, maxit=MAXIT):
    key = (nlab, maxit)
    if key not in _PROGRAM_CACHE:
        _PROGRAM_CACHE[key] = _build_program(nlab, maxit)
    return _PROGRAM_CACHE[key]


# --------------------------------------------------------------------------
# host driver
# --------------------------------------------------------------------------

def _softmax(x):
    x = np.asarray(x, np.float32)
    e = np.exp(x - x.max(axis=1, keepdims=True))
    return e / e.sum(axis=1, keepdims=True)


LAST_RESULTS = None


def prepare(features_0, features_1, features_2, features_3, embedds, outs,
            labels, labelled_idx, unlabelled_idx, ls_params, mlp_params,
            final_W, final_b):
    """Host-side input prep: returns (in_maps, perm, nlab_dev, lab)."""
    feats = [np.asarray(f, np.float32) for f in
             (features_0, features_1, features_2, features_3)]
    embedds = np.asarray(embedds, np.float32)
    outs = np.asarray(outs, np.float32)
    lab = np.asarray(labelled_idx).astype(np.int64)
    unlab = np.asarray(unlabelled_idx).astype(np.int64)

    in_lab = np.zeros(B, bool)
    in_lab[lab] = True
    perm = np.concatenate([np.nonzero(in_lab)[0], np.nonzero(~in_lab)[0]])
    nlab_dev = int(in_lab.sum())

    # X0 with the reference's scatter semantics, then permuted + tiled
    probs = _softmax(outs)
    X0 = np.zeros((B, NC_CLS), np.float32)
    X0[lab] = probs[lab]
    X0[unlab] = 1.0 / NC_CLS
    X0p = X0[perm]
    x0_dev = np.ascontiguousarray(
        X0p.reshape(NT, 128, NC_CLS).transpose(1, 0, 2).reshape(128, NT * NC_CLS))

    # embedds: host-normalized, transposed, bf16 (consumed only by core 4)
    en = embedds[perm]
    en = en / np.clip(np.linalg.norm(en, axis=1, keepdims=True), 1e-12, None)
    ebase4 = np.ascontiguousarray(en.T).astype(ml_dtypes.bfloat16).reshape(4, 128, B)
    ebase0 = np.zeros((4, 128, B), ml_dtypes.bfloat16)

    ident = np.eye(128, dtype=np.float32)

    wlist, blist = [], []
    for i in range(4):
        W, bias = ls_params[i]
        wlist.append(np.ascontiguousarray(
            np.asarray(W, np.float32) / (FSS[i] * FSS[i])))
        blist.append(np.ascontiguousarray(
            np.asarray(bias, np.float32).reshape(INTERM, 1)))

    def mlp_pack(br):
        layers = mlp_params[br]
        (W1, _b1, G1, BE1) = [np.asarray(a, np.float32) for a in layers[0]]
        d = {
            "mw1a": np.ascontiguousarray(W1[:128]),
            "mw1b": np.ascontiguousarray(W1[128:129]),
            "g1a": np.ascontiguousarray(G1[:128].reshape(128, 1)),
            "g1b": np.ascontiguousarray(G1[128:129].reshape(1, 1)),
            "be1a": np.ascontiguousarray(BE1[:128].reshape(128, 1)),
            "be1b": np.ascontiguousarray(BE1[128:129].reshape(1, 1)),
        }
        (W2, _b2, G2, BE2) = [np.asarray(a, np.float32) for a in layers[1]]
        d.update(mw2a=np.ascontiguousarray(W2[:128]),
                 mw2b=np.ascontiguousarray(W2[128:129]),
                 g2=np.ascontiguousarray(G2.reshape(-1, 1)),
                 be2=np.ascontiguousarray(BE2.reshape(-1, 1)))
        for li, nm in [(2, "3"), (3, "4"), (4, "5")]:
            (Wl, _bl, Gl, BEl) = [np.asarray(a, np.float32) for a in layers[li]]
            d["mw" + nm] = np.ascontiguousarray(Wl)
            d["g" + nm] = np.ascontiguousarray(Gl.reshape(-1, 1))
            d["be" + nm] = np.ascontiguousarray(BEl.reshape(-1, 1))
        return d

    finw_np = np.ascontiguousarray(np.asarray(final_W, np.float32).reshape(128, 1))
    finb_np = np.ascontiguousarray(np.asarray(final_b, np.float32).reshape(1, 1))

    featp = [np.take(f.reshape(B, -1), perm, axis=0) for f in feats]

    in_maps = []
    for c in range(NCORES):
        m = {}
        for i in range(4):
            m[f"f{i}"] = featp[i][BLOC * c:BLOC * (c + 1)]
            m[f"w{i}"] = wlist[i]
            m[f"b{i}"] = blist[i]
        m["ebase"] = ebase4 if c == 4 else ebase0
        am = np.zeros((128, 5), np.float32)
        if c < 4:
            am[:, c] = 1.0
        elif c == 4:
            am[:, 4] = 1.0
        m["amask"] = am
        m["x0"] = x0_dev
        m["ident"] = ident
        m.update(mlp_pack(c if c < 4 else 0))
        m["finw"] = finw_np
        m["finb"] = finb_np
        in_maps.append(m)

    return in_maps, perm, nlab_dev, lab


def kernel(features_0, features_1, features_2, features_3, embedds, outs,
           labels, labelled_idx, unlabelled_idx, ls_params, mlp_params,
           final_W, final_b, _maxit=MAXIT, _trace=False):
    in_maps, perm, nlab_dev, lab = prepare(
        features_0, features_1, features_2, features_3, embedds, outs,
        labels, labelled_idx, unlabelled_idx, ls_params, mlp_params,
        final_W, final_b)
    nc = _get_program(nlab_dev, _maxit)
    res = run_bass_kernel_spmd(nc, in_maps, core_ids=list(range(NCORES)),
                               trace=_trace)
    global LAST_RESULTS
    LAST_RESULTS = res
    outs_list = res.results

    inv = np.empty(B, np.int64)
    inv[perm] = np.arange(B)
    y_pred = np.asarray(outs_list[0]["y_pred"], np.float32)[inv]
    y_true = np.asarray(outs_list[4]["q_out"], np.float32)[inv]
    mask = np.zeros(B, np.float32)
    mask[lab] = 1.0
    return y_pred, y_true, mask.astype(bool)


# revision 21
# speedup vs baseline: 9.6456x; 9.2934x over previous
"""Trainium2 Bass kernel for nn_GTGModule (GTG message passing + MLP heads).

Self-contained: accepts FULL inputs (as produced by the problem's
setup_inputs), shards across 8 NeuronCores internally, returns FULL outputs
(y_pred, y_true, mask).

Strategy (single SPMD launch, 8 cores, 3 AllGathers):
  - batch-sharded feature pooling (the ~1GB memory-bound part)
  - host pre-permutes the batch so labelled rows are [0, nlab) => the
    labelled x labelled block of the affinity matrix is tile-contiguous
  - each of cores 0-3 runs one branch's full B x B GTG; core 4 runs the
    embedds GTG; selection is done with per-core 0/1 mask inputs (pure SPMD)
  - MLP runs with transposed activations so BatchNorm is a free-dim reduce
"""

import numpy as np
import ml_dtypes

import concourse.bass as bass
import concourse.bacc as bacc
import concourse.bass_isa as bass_isa
import concourse.mybir as mybir
from concourse import tile
from concourse.bass_utils import run_bass_kernel_spmd

F32 = mybir.dt.float32
BF16 = mybir.dt.bfloat16
ALU = mybir.AluOpType
ACTF = mybir.ActivationFunctionType
AX = mybir.AxisListType

B = 2048
NC_CLS = 10
NCORES = 8
BLOC = B // NCORES          # 256 batch rows per core
CHS = [64, 128, 256, 512]
FSS = [32, 16, 8, 4]
INTERM = 128
MAXIT = 30
TOL = 1e-3
BN_EPS = 1e-5
NT = B // 128               # 16 row blocks
DMA_CHUNK = 4096            # free elems per pooling DMA tile (16KB/partition)


# --------------------------------------------------------------------------
# device program
# --------------------------------------------------------------------------

def _build_program(nlab, maxit=MAXIT, solo=False):
    nc = bacc.Bacc("TRN2", num_devices=NCORES)

    # ---------------- inputs ----------------
    feats = [nc.dram_tensor(f"f{i}", [BLOC, CHS[i] * FSS[i] * FSS[i]], F32,
                            kind="ExternalInput") for i in range(4)]
    ws = [nc.dram_tensor(f"w{i}", [CHS[i], INTERM], F32, kind="ExternalInput")
          for i in range(4)]
    bs = [nc.dram_tensor(f"b{i}", [INTERM, 1], F32, kind="ExternalInput")
          for i in range(4)]
    ebase = nc.dram_tensor("ebase", [4, 128, B], BF16, kind="ExternalInput")
    amask_in = nc.dram_tensor("amask", [128, 5], F32, kind="ExternalInput")
    x0_in = nc.dram_tensor("x0", [128, NT * NC_CLS], F32, kind="ExternalInput")
    ident_in = nc.dram_tensor("ident", [128, 128], F32, kind="ExternalInput")
    w1a = nc.dram_tensor("mw1a", [128, 129], F32, kind="ExternalInput")
    w1b = nc.dram_tensor("mw1b", [1, 129], F32, kind="ExternalInput")
    w2a = nc.dram_tensor("mw2a", [128, 64], F32, kind="ExternalInput")
    w2b = nc.dram_tensor("mw2b", [1, 64], F32, kind="ExternalInput")
    w3 = nc.dram_tensor("mw3", [64, 64], F32, kind="ExternalInput")
    w4 = nc.dram_tensor("mw4", [64, 32], F32, kind="ExternalInput")
    w5 = nc.dram_tensor("mw5", [32, 32], F32, kind="ExternalInput")
    g1a = nc.dram_tensor("g1a", [128, 1], F32, kind="ExternalInput")
    g1b = nc.dram_tensor("g1b", [1, 1], F32, kind="ExternalInput")
    be1a = nc.dram_tensor("be1a", [128, 1], F32, kind="ExternalInput")
    be1b = nc.dram_tensor("be1b", [1, 1], F32, kind="ExternalInput")
    g2 = nc.dram_tensor("g2", [64, 1], F32, kind="ExternalInput")
    be2 = nc.dram_tensor("be2", [64, 1], F32, kind="ExternalInput")
    g3 = nc.dram_tensor("g3", [64, 1], F32, kind="ExternalInput")
    be3 = nc.dram_tensor("be3", [64, 1], F32, kind="ExternalInput")
    g4 = nc.dram_tensor("g4", [32, 1], F32, kind="ExternalInput")
    be4 = nc.dram_tensor("be4", [32, 1], F32, kind="ExternalInput")
    g5 = nc.dram_tensor("g5", [32, 1], F32, kind="ExternalInput")
    be5 = nc.dram_tensor("be5", [32, 1], F32, kind="ExternalInput")
    finw = nc.dram_tensor("finw", [128, 1], F32, kind="ExternalInput")
    finb = nc.dram_tensor("finb", [1, 1], F32, kind="ExternalInput")

    # ---------------- outputs ----------------
    ypred_out = nc.dram_tensor("y_pred", [B], F32, kind="ExternalOutput")
    q_out = nc.dram_tensor("q_out", [B], F32, kind="ExternalOutput")

    # ---------------- internal DRAM ----------------
    ag1n_in = nc.dram_tensor("ag1n_in", [4, 128, BLOC], BF16, kind="Internal")
    ag1n_out = nc.dram_tensor("ag1n_out", [NCORES, 4, 128, BLOC], BF16,
                              kind="Internal", addr_space="Shared")
    ag1r_in = nc.dram_tensor("ag1r_in", [4, 128, BLOC], F32, kind="Internal")
    ag1r_out = nc.dram_tensor("ag1r_out", [NCORES, 4, 128, BLOC], F32,
                              kind="Internal", addr_space="Shared")
    ag2_in = nc.dram_tensor("ag2_in", [32, B], F32, kind="Internal")
    ag2_out = nc.dram_tensor("ag2_out", [NCORES, 32, B], F32,
                             kind="Internal", addr_space="Shared")
    eraw_dram = nc.dram_tensor("eraw_dram", [128, B], F32, kind="Internal")
    q_dram = nc.dram_tensor("q_dram", [B], F32, kind="Internal")
    rg = [list(range(NCORES))]

    with tile.TileContext(nc) as tc:
        with tc.tile_pool(name="persist", bufs=1) as pp, \
             tc.tile_pool(name="a16", bufs=1) as a16pool, \
             tc.tile_pool(name="psmall", bufs=2, space="PSUM") as psmall:

            ident = pp.tile([128, 128], F32, tag="ident")
            nc.sync.dma_start(ident[:], ident_in[:, :])
            amask = pp.tile([128, 5], F32, tag="amask")
            nc.sync.dma_start(amask[:], amask_in[:, :])
            ones_col = pp.tile([128, 1], F32, tag="ones_col")
            nc.vector.memset(ones_col[:], 1.0)
            ones_row = pp.tile([1, 128], F32, tag="ones_row")
            nc.vector.memset(ones_row[:], 1.0)
            ones10 = pp.tile([10, 1], F32, tag="ones10")
            nc.vector.memset(ones10[:], 1.0)

            # ============= phase 1: pooling + branch embeddings =============
            with tc.tile_pool(name="pool_dma", bufs=4) as fpool, \
                 tc.tile_pool(name="pool_rm", bufs=2) as prpool, \
                 tc.tile_pool(name="pool_w", bufs=1) as wpool, \
                 tc.tile_pool(name="pooledT", bufs=1) as ptpool, \
                 tc.tile_pool(name="emb", bufs=1) as embpool, \
                 tc.tile_pool(name="pool_ps", bufs=2, space="PSUM") as ppsum, \
                 tc.tile_pool(name="emb_ps", bufs=2, space="PSUM") as epsum:
                embraw = []   # (128, BLOC) f32, per branch
                ntpart = []   # (128, BLOC) bf16, per branch (row-normalized)
                for i in range(4):
                    CH, S = CHS[i], FSS[i] * FSS[i]
                    row = CH * S
                    nchunks = row // DMA_CHUNK
                    chpc = DMA_CHUNK // S          # channels per chunk
                    nkchunk = (CH + 127) // 128    # partition chunks of pooledT
                    ptiles = [ptpool.tile([128, BLOC], F32, tag=f"pt{i}_{j}",
                                          name=f"pt{i}_{j}")
                              for j in range(nkchunk)]
                    for bt in range(BLOC // 128):
                        pr = prpool.tile([128, CH], F32, tag="poolrm")
                        for c in range(nchunks):
                            ft = fpool.tile([128, DMA_CHUNK], F32, tag="ftile")
                            nc.sync.dma_start(
                                ft[:],
                                feats[i][128 * bt:128 * (bt + 1),
                                         DMA_CHUNK * c:DMA_CHUNK * (c + 1)])
                            nc.vector.tensor_reduce(
                                pr[:, chpc * c:chpc * (c + 1)],
                                ft[:].rearrange("p (ch s) -> p ch s", s=S),
                                axis=AX.X, op=ALU.add)
                        # transpose (128, CH) -> (CH, 128) into pooledT chunks
                        for j in range(nkchunk):
                            w = min(128, CH - 128 * j)
                            tp = ppsum.tile([128, 128], F32, tag="tpsum")
                            nc.tensor.transpose(
                                tp[:w, :], pr[:, 128 * j:128 * j + w], ident[:])
                            nc.scalar.copy(
                                ptiles[j][:w, 128 * bt:128 * (bt + 1)], tp[:w, :])
                    # emb = relu(W.T @ pooledT + bias): out (128, BLOC)
                    kp = min(128, CH)
                    wsb = wpool.tile([128, nkchunk * INTERM], F32, tag=f"wsb{i}")
                    nc.sync.dma_start(
                        wsb[:kp, :].rearrange("p (k m) -> p k m", m=INTERM),
                        ws[i][:, :].rearrange("(k p) m -> p k m", p=kp))
                    bsb = wpool.tile([128, 1], F32, tag=f"bsb{i}")
                    nc.sync.dma_start(bsb[:], bs[i][:, :])
                    eps = epsum.tile([128, BLOC], F32, tag="embps")
                    for j in range(nkchunk):
                        w = min(128, CH - 128 * j)
                        nc.tensor.matmul(
                            eps[:, :], wsb[:w, INTERM * j:INTERM * (j + 1)],
                            ptiles[j][:w, :],
                            start=(j == 0), stop=(j == nkchunk - 1))
                    er = embpool.tile([128, BLOC], F32, tag=f"embraw{i}")
                    nc.scalar.activation(er[:], eps[:], ACTF.Relu, bias=bsb[:, 0:1])
                    embraw.append(er)
                    # row-normalize (norm over the 128 channels = partition dim)
                    sq = prpool.tile([128, BLOC], F32, tag="sqscratch")
                    nc.scalar.activation(sq[:], er[:], ACTF.Square)
                    n2 = psmall.tile([1, BLOC], F32, tag="ps")
                    nc.tensor.matmul(n2[:], ones_col[:], sq[:], start=True, stop=True)
                    nrm = prpool.tile([1, BLOC], F32, tag="nrm")
                    nc.scalar.activation(nrm[:], n2[:], ACTF.Sqrt)
                    nc.vector.tensor_scalar(nrm[:], nrm[:], 1e-12, None, op0=ALU.max)
                    rn = prpool.tile([1, BLOC], F32, tag="rn")
                    nc.vector.reciprocal(rn[:], nrm[:])
                    rnb = psmall.tile([128, BLOC], F32, tag="ps")
                    nc.tensor.matmul(rnb[:], ones_row[:], rn[:], start=True, stop=True)
                    nt_ = embpool.tile([128, BLOC], BF16, tag=f"ntpart{i}")
                    nc.vector.tensor_tensor(nt_[:], er[:], rnb[:], op=ALU.mult)
                    ntpart.append(nt_)

                # ===== phase 2: AllGather emb parts =====
                for i in range(4):
                    nc.sync.dma_start(ag1n_in[i, :, :], ntpart[i][:])
                    nc.sync.dma_start(ag1r_in[i, :, :], embraw[i][:])
            if solo:
                nc.sync.dma_start(ag1n_out[0, :, :, :], ag1n_in[:, :, :])
                nc.sync.dma_start(ag1r_out[0, :, :, :], ag1r_in[:, :, :])
            else:
                nc.gpsimd.collective_compute(
                    "AllGather", ALU.bypass, replica_groups=rg,
                    ins=[ag1n_in[:, :, :].opt()],
                    outs=[ag1n_out[:, :, :, :].opt()])
                nc.gpsimd.collective_compute(
                    "AllGather", ALU.bypass, replica_groups=rg,
                    ins=[ag1r_in[:, :, :].opt()],
                    outs=[ag1r_out[:, :, :, :].opt()])

            # ============= phase 3: blend per-core E, build A =============
            a16 = [a16pool.tile([128, B], BF16, tag=f"a16_{t}", name=f"a16_{t}")
                   for t in range(NT)]
            rowsums = pp.tile([128, 4 * NT], F32, tag="rowsums")
            if True:
                with tc.tile_pool(name="enorm", bufs=1) as enpool:
                    enorm = [enpool.tile([128, B], BF16, tag=f"en{q}", name=f"en{q}")
                             for q in range(4)]
                    with tc.tile_pool(name="blend", bufs=2) as blpool:
                        # raw E blend (chunk 0 only; branch cores only)
                        eraw = blpool.tile([128, B], F32, tag="erawblend")
                        for i in range(4):
                            rb = blpool.tile([128, B], F32, tag="ldtile")
                            nc.sync.dma_start(
                                rb[:].rearrange("p (r b) -> p r b", b=BLOC),
                                ag1r_out[:, i, :, :].rearrange("r p b -> p r b"))
                            if i == 0:
                                nc.vector.tensor_scalar(
                                    eraw[:], rb[:], amask[:, 0:1], None,
                                    op0=ALU.mult)
                            else:
                                nc.vector.scalar_tensor_tensor(
                                    eraw[:], rb[:], amask[:, i:i + 1], eraw[:],
                                    op0=ALU.mult, op1=ALU.add)
                        nc.sync.dma_start(eraw_dram[:, :], eraw[:])
                        # normalized E blend: chunk 0 from branches + ebase
                        for i in range(4):
                            nb = blpool.tile([128, B], BF16, tag="ldtile")
                            nc.sync.dma_start(
                                nb[:].rearrange("p (r b) -> p r b", b=BLOC),
                                ag1n_out[:, i, :, :].rearrange("r p b -> p r b"))
                            if i == 0:
                                nc.vector.tensor_scalar(
                                    enorm[0][:], nb[:], amask[:, 0:1], None,
                                    op0=ALU.mult)
                            else:
                                nc.vector.scalar_tensor_tensor(
                                    enorm[0][:], nb[:], amask[:, i:i + 1],
                                    enorm[0][:], op0=ALU.mult, op1=ALU.add)
                        eb = blpool.tile([128, B], BF16, tag="ldtile")
                        nc.sync.dma_start(eb[:], ebase[0, :, :])
                        nc.vector.scalar_tensor_tensor(
                            enorm[0][:], eb[:], amask[:, 4:5], enorm[0][:],
                            op0=ALU.mult, op1=ALU.add)
                        for qch in range(1, 4):
                            ebq = blpool.tile([128, B], BF16, tag="ldtile")
                            nc.sync.dma_start(ebq[:], ebase[qch, :, :])
                            nc.vector.tensor_scalar(
                                enorm[qch][:], ebq[:], amask[:, 4:5], None,
                                op0=ALU.mult)

                    # A = relu(E^T E) with fused fp32 row-sums, stored bf16
                    with tc.tile_pool(name="aps", bufs=4, space="PSUM") as apsum:
                        for t in range(NT):
                            for n in range(4):
                                ps = apsum.tile([128, 512], F32, tag="aps")
                                for k in range(4):
                                    nc.tensor.matmul(
                                        ps[:],
                                        enorm[k][:, 128 * t:128 * (t + 1)],
                                        enorm[k][:, 512 * n:512 * (n + 1)],
                                        start=(k == 0), stop=(k == 3))
                                nc.scalar.activation(
                                    a16[t][:, 512 * n:512 * (n + 1)], ps[:],
                                    ACTF.Relu,
                                    accum_out=rowsums[:, 4 * t + n:4 * t + n + 1])
                # enorm freed here; mean: m = (S - B) / B^2
                rs2 = pp.tile([128, 1], F32, tag="rs2")
                nc.vector.tensor_reduce(rs2[:], rowsums[:], axis=AX.X, op=ALU.add)
                sps = psmall.tile([1, 1], F32, tag="ps")
                nc.tensor.matmul(sps[:], ones_col[:], rs2[:], start=True, stop=True)
                m_sb = pp.tile([1, 1], F32, tag="m_sb")
                nc.scalar.activation(m_sb[:], sps[:], ACTF.Copy,
                                     bias=-float(B) / (B * B), scale=1.0 / (B * B))
                mps = psmall.tile([128, 1], F32, tag="ps")
                nc.tensor.matmul(mps[:], ones_row[:], m_sb[:], start=True, stop=True)
                m128 = pp.tile([128, 1], F32, tag="m128")
                nc.scalar.copy(m128[:], mps[:])

                # threshold (in place) + flip (labelled rows/cols = [0, nlab))
                for t in range(NT):
                    eng = nc.vector
                    eng.scalar_tensor_tensor(
                        a16[t][:], a16[t][:], m128[:, 0:1], a16[t][:],
                        op0=ALU.is_ge, op1=ALU.mult)
                    lr = min(max(nlab - 128 * t, 0), 128)  # labelled rows here
                    # flip (x -> 1 - x) everywhere except lab-rows x lab-cols
                    if lr > 0 and nlab < B:
                        nc.scalar.activation(
                            a16[t][:lr, nlab:B], a16[t][:lr, nlab:B],
                            ACTF.Copy, bias=1.0, scale=-1.0)
                    if lr < 128:
                        nc.scalar.activation(
                            a16[t][lr:, :], a16[t][lr:, :],
                            ACTF.Copy, bias=1.0, scale=-1.0)
                    # diagonal: labelled rows -> 0, unlabelled rows -> 1
                    if lr > 0:
                        nc.gpsimd.affine_select(
                            a16[t][:lr, 128 * t:128 * (t + 1)],
                            a16[t][:lr, 128 * t:128 * (t + 1)],
                            pattern=[[1, 128]], compare_op=ALU.not_equal,
                            fill=0.0, base=0, channel_multiplier=-1)
                    if lr < 128:
                        nc.gpsimd.affine_select(
                            a16[t][lr:, 128 * t:128 * (t + 1)],
                            a16[t][lr:, 128 * t:128 * (t + 1)],
                            pattern=[[1, 128]], compare_op=ALU.not_equal,
                            fill=1.0, base=-lr, channel_multiplier=-1)

            # ============= phase 4: GTG iterations =============
            FD = NT * NC_CLS  # 160
            x32 = pp.tile([128, FD], F32, tag="x32")
            nc.sync.dma_start(x32[:], x0_in[:, :])
            xb = pp.tile([128, FD], BF16, tag="xb")
            nc.vector.tensor_copy(xb[:], x32[:])
            active = pp.tile([128, 1], F32, tag="active")
            nc.vector.memset(active[:], 1.0)
            entacc = pp.tile([128, NT], F32, tag="entacc")
            nc.vector.memset(entacc[:], 0.0)

            with tc.tile_pool(name="gtg", bufs=2) as gt, \
                 tc.tile_pool(name="gtg_ps", bufs=2, space="PSUM") as gtps:
                for it in range(maxit):
                    last = (it == maxit - 1)
                    # Y^T = X^T A  (X slices stationary: 10-col weight loads;
                    # A streams at N=512), then PE-transpose back to (128, FD)
                    yt_sb = gt.tile([10, B], F32, tag="yt_sb")
                    for half in range(2):
                        ytp = gtps.tile([10, B // 2], F32, tag=f"yt{half}", bufs=1)
                        for k in range(NT):
                            for n2 in range(2):
                                c0 = 1024 * half + 512 * n2
                                nc.tensor.matmul(
                                    ytp[:, 512 * n2:512 * (n2 + 1)],
                                    xb[:, 10 * k:10 * (k + 1)],
                                    a16[k][:, c0:c0 + 512],
                                    start=(k == 0), stop=(k == NT - 1))
                        nc.scalar.copy(
                            yt_sb[:, 1024 * half:1024 * (half + 1)], ytp[:])
                    yps = gtps.tile([128, FD], F32, tag="yps")
                    for t in range(NT):
                        nc.tensor.transpose(
                            yps[:, 10 * t:10 * (t + 1)],
                            yt_sb[:, 128 * t:128 * (t + 1)],
                            ident[:10, :10])
                    # --- critical chain to X_next (DVE FIFO order matters) ---
                    msb = gt.tile([128, FD], F32, tag="msb")
                    nc.vector.tensor_tensor(msb[:], x32[:], yps[:], op=ALU.mult)
                    r = gt.tile([128, NT], F32, tag="r")
                    nc.vector.tensor_reduce(
                        r[:], msb[:].rearrange("p (t c) -> p t c", c=NC_CLS),
                        axis=AX.X, op=ALU.add)
                    rinv = gt.tile([128, NT], F32, tag="rinv")
                    nc.vector.reciprocal(rinv[:], r[:])
                    xn = gt.tile([128, FD], F32, tag="xn")
                    nc.vector.tensor_tensor(
                        xn[:].rearrange("p (t c) -> p t c", c=NC_CLS),
                        msb[:].rearrange("p (t c) -> p t c", c=NC_CLS),
                        rinv[:, :, None].to_broadcast((128, NT, NC_CLS)),
                        op=ALU.mult)
                    if not last:
                        d = gt.tile([128, FD], F32, tag="d")
                        nc.vector.tensor_tensor(d[:], xn[:], x32[:],
                                                op=ALU.subtract)
                        # X_next = X + active_old * (Xn - X)  (OLD active)
                        nc.vector.scalar_tensor_tensor(
                            x32[:], d[:], active[:, 0:1], x32[:],
                            op0=ALU.mult, op1=ALU.add)
                        nc.vector.tensor_copy(xb[:], x32[:])
                    # --- entropy / stop-flag (off the critical path) ---
                    xc = gt.tile([128, FD], F32, tag="xc")
                    nc.vector.tensor_scalar(xc[:], xn[:], 1e-8, 1.0,
                                            op0=ALU.max, op1=ALU.min)
                    lg = gt.tile([128, FD], F32, tag="lg")
                    nc.scalar.activation(lg[:], xc[:], ACTF.Ln)
                    pq = gt.tile([128, FD], F32, tag="pq")
                    nc.gpsimd.tensor_tensor(pq[:], xc[:], lg[:], op=ALU.mult)
                    entsum = gt.tile([128, NT], F32, tag="entsum")
                    nc.vector.tensor_reduce(
                        entsum[:], pq[:].rearrange("p (t c) -> p t c", c=NC_CLS),
                        axis=AX.X, op=ALU.add)
                    nc.vector.scalar_tensor_tensor(
                        entacc[:], entsum[:], active[:, 0:1], entacc[:],
                        op0=ALU.mult, op1=ALU.add)
                    if not last:
                        dsq_s = gt.tile([128, FD], F32, tag="dsq_s")
                        dsq = gt.tile([128, 1], F32, tag="dsq")
                        nc.scalar.activation(dsq_s[:], d[:], ACTF.Square,
                                             accum_out=dsq[:])
                        # cross-partition err reduction on GPSIMD: keeps the
                        # PE FIFO free of late-dependency matmuls
                        errall = gt.tile([128, 1], F32, tag="errall")
                        nc.gpsimd.partition_all_reduce(
                            errall[:], dsq[:], channels=128,
                            reduce_op=bass_isa.ReduceOp.add)
                        flag = gt.tile([128, 1], F32, tag="flag")
                        nc.vector.tensor_scalar(flag[:], errall[:], TOL * TOL,
                                                None, op0=ALU.is_gt)
                        # update active for the next iteration (after uses)
                        nc.vector.tensor_tensor(active[:], active[:], flag[:],
                                                op=ALU.mult)

            # q = -entacc / maxit   (layout (p, t): device row = 128 t + p)
            q128 = pp.tile([128, NT], F32, tag="q128")
            nc.vector.tensor_scalar(q128[:], entacc[:], -1.0 / maxit, None,
                                    op0=ALU.mult)
            nc.sync.dma_start(
                q_dram[:].rearrange("(t p) -> p t", p=128), q128[:])
            nc.sync.dma_start(
                q_out[:].rearrange("(t p) -> p t", p=128), q128[:])

            # ============= phase 5: MLP (transposed activations) =============
            def bn_relu(h_ps, P, g_dram, be_dram, out_sb, sp):
                """out = relu(batchnorm(h) * g + be); h_ps: (P, B) psum/sbuf."""
                rowsum = sp.tile([128, 1], F32, tag="bn_rowsum")
                nc.vector.tensor_reduce(rowsum[:P, :], h_ps[:P, :],
                                        axis=AX.X, op=ALU.add)
                sqs = sp.tile([128, B], F32, tag="bn_sqs")
                sq2 = sp.tile([128, 1], F32, tag="bn_sq2")
                nc.scalar.activation(sqs[:P, :], h_ps[:P, :], ACTF.Square,
                                     accum_out=sq2[:P, :])
                mu = sp.tile([128, 1], F32, tag="bn_mu")
                nc.vector.tensor_scalar(mu[:P, :], rowsum[:P, :], 1.0 / B, None,
                                        op0=ALU.mult)
                ex2 = sp.tile([128, 1], F32, tag="bn_ex2")
                nc.vector.tensor_scalar(ex2[:P, :], sq2[:P, :], 1.0 / B, None,
                                        op0=ALU.mult)
                musq = sp.tile([128, 1], F32, tag="bn_musq")
                nc.vector.tensor_tensor(musq[:P, :], mu[:P, :], mu[:P, :],
                                        op=ALU.mult)
                var = sp.tile([128, 1], F32, tag="bn_var")
                nc.vector.tensor_tensor(var[:P, :], ex2[:P, :], musq[:P, :],
                                        op=ALU.subtract)
                nc.vector.tensor_scalar(var[:P, :], var[:P, :], BN_EPS, None,
                                        op0=ALU.add)
                std = sp.tile([128, 1], F32, tag="bn_std")
                nc.scalar.activation(std[:P, :], var[:P, :], ACTF.Sqrt)
                rstd = sp.tile([128, 1], F32, tag="bn_rstd")
                nc.vector.reciprocal(rstd[:P, :], std[:P, :])
                gt_ = sp.tile([128, 1], F32, tag="bn_g")
                nc.sync.dma_start(gt_[:P, :], g_dram[:, :])
                bt_ = sp.tile([128, 1], F32, tag="bn_be")
                nc.sync.dma_start(bt_[:P, :], be_dram[:, :])
                s = sp.tile([128, 1], F32, tag="bn_s")
                nc.vector.tensor_tensor(s[:P, :], gt_[:P, :], rstd[:P, :],
                                        op=ALU.mult)
                nmu = sp.tile([128, 1], F32, tag="bn_nmu")
                nc.vector.tensor_scalar(nmu[:P, :], mu[:P, :], -1.0, None,
                                        op0=ALU.mult)
                bb = sp.tile([128, 1], F32, tag="bn_bb")
                nc.vector.scalar_tensor_tensor(
                    bb[:P, :], nmu[:P, :], s[:P, 0:1], bt_[:P, :],
                    op0=ALU.mult, op1=ALU.add)
                nc.scalar.activation(out_sb[:P, :], h_ps[:P, :], ACTF.Relu,
                                     bias=bb[:P, 0:1], scale=s[:P, 0:1])

            with tc.tile_pool(name="mlp", bufs=1) as mp, \
                 tc.tile_pool(name="mlp_w", bufs=1) as mw, \
                 tc.tile_pool(name="mlp_hps", bufs=1, space="PSUM") as hpool, \
                 tc.tile_pool(name="mlp_xps", bufs=2, space="PSUM") as xpool, \
                 tc.tile_pool(name="mlp_s", bufs=2) as msc:
                er2 = mp.tile([128, B], F32, tag="er2")
                nc.sync.dma_start(er2[:], eraw_dram[:, :])
                qT = mp.tile([1, B], F32, tag="qT")
                nc.sync.dma_start(qT[:], q_dram[:].rearrange("(a b) -> a b", a=1))
                w1a_sb = mw.tile([128, 129], F32, tag="mw1a")
                nc.sync.dma_start(w1a_sb[:], w1a[:, :])
                w1b_sb = mw.tile([1, 129], F32, tag="mw1b")
                nc.sync.dma_start(w1b_sb[:], w1b[:, :])
                # layer 1 main (out rows 0..127)
                h1 = hpool.tile([128, B], F32, tag="hbig")
                for n in range(4):
                    sl = slice(512 * n, 512 * (n + 1))
                    nc.tensor.matmul(h1[:, sl], w1a_sb[:, 0:128], er2[:, sl],
                                     start=True, stop=False)
                    nc.tensor.matmul(h1[:, sl], w1b_sb[:, 0:128], qT[:, sl],
                                     start=False, stop=True)
                z1 = mp.tile([128, B], F32, tag="z1")
                bn_relu(h1, 128, g1a, be1a, z1, msc)
                # layer 1 extra row (out row 128) via chunked psum -> sbuf
                h1x_sb = mp.tile([1, B], F32, tag="h1x_sb")
                for n in range(4):
                    sl = slice(512 * n, 512 * (n + 1))
                    hx = xpool.tile([1, 512], F32, tag="hx")
                    nc.tensor.matmul(hx[:], w1a_sb[:, 128:129], er2[:, sl],
                                     start=True, stop=False)
                    nc.tensor.matmul(hx[:], w1b_sb[:, 128:129], qT[:, sl],
                                     start=False, stop=True)
                    nc.scalar.copy(h1x_sb[:, sl], hx[:])
                z1x = mp.tile([1, B], F32, tag="z1x")
                bn_relu(h1x_sb, 1, g1b, be1b, z1x, msc)
                # layer 2: (129 -> 64)
                w2a_sb = mw.tile([128, 64], F32, tag="mw2a")
                nc.sync.dma_start(w2a_sb[:], w2a[:, :])
                w2b_sb = mw.tile([1, 64], F32, tag="mw2b")
                nc.sync.dma_start(w2b_sb[:], w2b[:, :])
                h2 = hpool.tile([64, B], F32, tag="hbig")
                for n in range(4):
                    sl = slice(512 * n, 512 * (n + 1))
                    nc.tensor.matmul(h2[:, sl], w2a_sb[:, :], z1[:, sl],
                                     start=True, stop=False)
                    nc.tensor.matmul(h2[:, sl], w2b_sb[:, :], z1x[:, sl],
                                     start=False, stop=True)
                z2 = mp.tile([64, B], F32, tag="z2")
                bn_relu(h2, 64, g2, be2, z2, msc)
                # layers 3-5
                zprev, pprev = z2, 64
                for (wd, gd, bed, dout, lname) in [
                        (w3, g3, be3, 64, "3"), (w4, g4, be4, 32, "4"),
                        (w5, g5, be5, 32, "5")]:
                    wsb_ = mw.tile([128, dout], F32, tag=f"wl{lname}")
                    nc.sync.dma_start(wsb_[:pprev, :], wd[:, :])
                    hp = hpool.tile([dout, B], F32, tag="hbig")
                    for n in range(4):
                        sl = slice(512 * n, 512 * (n + 1))
                        nc.tensor.matmul(hp[:, sl], wsb_[:pprev, :],
                                         zprev[:pprev, sl],
                                         start=True, stop=True)
                    znew = mp.tile([dout, B], F32, tag=f"z{lname}")
                    bn_relu(hp, dout, gd, bed, znew, msc)
                    zprev, pprev = znew, dout

                # ===== phase 6: gather branch outputs, final head =====
                nc.sync.dma_start(ag2_in[:, :], zprev[:32, :])
                if solo:
                    nc.sync.dma_start(ag2_out[0, :, :], ag2_in[:, :])
                else:
                    nc.gpsimd.collective_compute(
                        "AllGather", ALU.bypass, replica_groups=rg,
                        ins=[ag2_in[:, :].opt()], outs=[ag2_out[:, :, :].opt()])
                zcat = mp.tile([128, B], F32, tag="zcat")
                nc.sync.dma_start(
                    zcat[:],
                    ag2_out[0:4, :, :].rearrange("r c b -> (r c) b"))
                fw = mw.tile([128, 1], F32, tag="fw")
                nc.sync.dma_start(fw[:], finw[:, :])
                fb = mw.tile([1, 1], F32, tag="fb")
                nc.sync.dma_start(fb[:], finb[:, :])
                yps_sb = mp.tile([1, B], F32, tag="ypsb")
                for n in range(4):
                    sl = slice(512 * n, 512 * (n + 1))
                    yp = xpool.tile([1, 512], F32, tag="hx")
                    nc.tensor.matmul(yp[:], fw[:], zcat[:, sl],
                                     start=True, stop=True)
                    nc.scalar.activation(yps_sb[:, sl], yp[:], ACTF.Identity,
                                         bias=fb[:, 0:1])
                nc.sync.dma_start(
                    ypred_out[:].rearrange("(a b) -> a b", a=1), yps_sb[:])

    nc.finalize()
    return nc


_PROGRAM_CACHE = {}


def _get_program(nlab, maxit=MAXIT):
    key = (nlab, maxit)
    if key not in _PROGRAM_CACHE:
        _PROGRAM_CACHE[key] = _build_program(nlab, maxit)
    return _PROGRAM_CACHE[key]


# --------------------------------------------------------------------------
# host driver
# --------------------------------------------------------------------------

def _softmax(x):
    x = np.asarray(x, np.float32)
    e = np.exp(x - x.max(axis=1, keepdims=True))
    return e / e.sum(axis=1, keepdims=True)


LAST_RESULTS = None


def prepare(features_0, features_1, features_2, features_3, embedds, outs,
            labels, labelled_idx, unlabelled_idx, ls_params, mlp_params,
            final_W, final_b):
    """Host-side input prep: returns (in_maps, perm, nlab_dev, lab)."""
    feats = [np.asarray(f, np.float32) for f in
             (features_0, features_1, features_2, features_3)]
    embedds = np.asarray(embedds, np.float32)
    outs = np.asarray(outs, np.float32)
    lab = np.asarray(labelled_idx).astype(np.int64)
    unlab = np.asarray(unlabelled_idx).astype(np.int64)

    in_lab = np.zeros(B, bool)
    in_lab[lab] = True
    perm = np.concatenate([np.nonzero(in_lab)[0], np.nonzero(~in_lab)[0]])
    nlab_dev = int(in_lab.sum())

    # X0 with the reference's scatter semantics, then permuted + tiled
    probs = _softmax(outs)
    X0 = np.zeros((B, NC_CLS), np.float32)
    X0[lab] = probs[lab]
    X0[unlab] = 1.0 / NC_CLS
    X0p = X0[perm]
    x0_dev = np.ascontiguousarray(
        X0p.reshape(NT, 128, NC_CLS).transpose(1, 0, 2).reshape(128, NT * NC_CLS))

    # embedds: host-normalized, transposed, bf16 (consumed only by core 4)
    en = embedds[perm]
    en = en / np.clip(np.linalg.norm(en, axis=1, keepdims=True), 1e-12, None)
    ebase4 = np.ascontiguousarray(en.T).astype(ml_dtypes.bfloat16).reshape(4, 128, B)
    ebase0 = np.zeros((4, 128, B), ml_dtypes.bfloat16)

    ident = np.eye(128, dtype=np.float32)

    wlist, blist = [], []
    for i in range(4):
        W, bias = ls_params[i]
        wlist.append(np.ascontiguousarray(
            np.asarray(W, np.float32) / (FSS[i] * FSS[i])))
        blist.append(np.ascontiguousarray(
            np.asarray(bias, np.float32).reshape(INTERM, 1)))

    def mlp_pack(br):
        layers = mlp_params[br]
        (W1, _b1, G1, BE1) = [np.asarray(a, np.float32) for a in layers[0]]
        d = {
            "mw1a": np.ascontiguousarray(W1[:128]),
            "mw1b": np.ascontiguousarray(W1[128:129]),
            "g1a": np.ascontiguousarray(G1[:128].reshape(128, 1)),
            "g1b": np.ascontiguousarray(G1[128:129].reshape(1, 1)),
            "be1a": np.ascontiguousarray(BE1[:128].reshape(128, 1)),
            "be1b": np.ascontiguousarray(BE1[128:129].reshape(1, 1)),
        }
        (W2, _b2, G2, BE2) = [np.asarray(a, np.float32) for a in layers[1]]
        d.update(mw2a=np.ascontiguousarray(W2[:128]),
                 mw2b=np.ascontiguousarray(W2[128:129]),
                 g2=np.ascontiguousarray(G2.reshape(-1, 1)),
                 be2=np.ascontiguousarray(BE2.reshape(-1, 1)))
        for li, nm in [(2, "3"), (3, "4"), (4, "5")]:
            (Wl, _bl, Gl, BEl) = [np.asarray(a, np.float32) for a in layers[li]]
            d["mw" + nm] = np.ascontiguousarray(Wl)
            d["g" + nm] = np.ascontiguousarray(Gl.reshape(-1, 1))
            d["be" + nm] = np.ascontiguousarray(BEl.reshape(-1, 1))
        return d

    finw_np = np.ascontiguousarray(np.asarray(final_W, np.float32).reshape(128, 1))
    finb_np = np.ascontiguousarray(np.asarray(final_b, np.float32).reshape(1, 1))

    featp = [np.take(f.reshape(B, -1), perm, axis=0) for f in feats]

    in_maps = []
    for c in range(NCORES):
        m = {}
        for i in range(4):
            m[f"f{i}"] = featp[i][BLOC * c:BLOC * (c + 1)]
            m[f"w{i}"] = wlist[i]
            m[f"b{i}"] = blist[i]
        m["ebase"] = ebase4 if c == 4 else ebase0
        am = np.zeros((128, 5), np.float32)
        if c < 4:
            am[:, c] = 1.0
        elif c == 4:
            am[:, 4] = 1.0
        m["amask"] = am
        m["x0"] = x0_dev
        m["ident"] = ident
        m.update(mlp_pack(c if c < 4 else 0))
        m["finw"] = finw_np
        m["finb"] = finb_np
        in_maps.append(m)

    return in_maps, perm, nlab_dev, lab


def kernel(features_0, features_1, features_2, features_3, embedds, outs,
           labels, labelled_idx, unlabelled_idx, ls_params, mlp_params,
           final_W, final_b, _maxit=MAXIT, _trace=False):
    in_maps, perm, nlab_dev, lab = prepare(
        features_0, features_1, features_2, features_3, embedds, outs,
        labels, labelled_idx, unlabelled_idx, ls_params, mlp_params,
        final_W, final_b)
    nc = _get_program(nlab_dev, _maxit)
    res = run_bass_kernel_spmd(nc, in_maps, core_ids=list(range(NCORES)),
                               trace=_trace)
    global LAST_RESULTS
    LAST_RESULTS = res
    outs_list = res.results

    inv = np.empty(B, np.int64)
    inv[perm] = np.arange(B)
    y_pred = np.asarray(outs_list[0]["y_pred"], np.float32)[inv]
    y_true = np.asarray(outs_list[4]["q_out"], np.float32)[inv]
    mask = np.zeros(B, np.float32)
    mask[lab] = 1.0
    return y_pred, y_true, mask.astype(bool)
